# revision 13
# baseline (speedup 1.0000x reference)
"""Bass/Trainium2 kernel for a heterogeneous-graph SAGEConv layer (DBGNNLayer).

Strategy: shard by DESTINATION node across the 8 cores (12,500 dst rows of
each node type per core) so no cross-core collectives are needed.  Within a
core, dst rows are packed into 100 windows of 128 rows each, using
load-balanced binning so that every (window, src-chunk) edge segment fits a
fixed capacity (SPMD-uniform static shapes).  Edge source rows are fetched
with dma_gather (int16 indices -> the 100k-row tables are split into 4
chunks of 25k rows) in fp16 (halves HBM gather bytes vs f32).  The
per-window segment mean is computed as a matmul with a scaled one-hot
matrix built on the vector engine in fp16:
    psum_msgT[fin, dstlocal] += Xg_tile[e, fin]^T @ onehot[e, dstlocal]
where onehot[e, d] = (iota[d] == dst_local[e]) * recip[e], recip folding in
the 1/deg mean and the HeteroConv 0.5.  The root term x_dst @ Wr reads a
host-pre-permuted x_dst table with a transposing HWDGE DMA (no gather, no
PE transpose).  The bias is folded into the PSUM accumulation as a rank-1
matmul (ones ⊗ b).  Final per window:
    out[dst, f] = msgT.T @ Wl (+ msgT_tags.T @ Wl_tags) + xdT.T @ Wr + 1⊗b
"""

import sys
import time

sys.path.insert(0, "/opt/trn_rl_repo")

import numpy as np

P = 128                 # partitions / feature dim / window rows
NC_CORES = 8
NW = 100                # windows per node type per core
S_CHUNK = 25000         # rows per gather chunk (int16-safe)
GRP_U = 20              # windows per gather group, user phase
GRP_I = 10              # windows per gather group, item phase

_COMPILED_CACHE = {}

# classed per-window capacities: NH heavy windows, NW-NH light.
# (NH, caph4, capl4, caph1, capl1): rev/buys per-chunk caps; tags caps.
_CAP_CONFIGS = [
    (60, 384, 256, 1408, 1152),       # classed (preferred)
    (100, 384, 384, 1280, 1280),      # uniform fallback
    (100, 512, 512, 1408, 1408),      # enlarged fallback
]


# ----------------------------------------------------------------- host utils

def _wrap16(flat_idx):
    """[n] int -> [128, n//16] int16 wrapped in 16 partitions, replicated."""
    n = flat_idx.shape[0]
    assert n % 16 == 0
    base = flat_idx.reshape(n // 16, 16).T.astype(np.int16)  # [16, n//16]
    return np.tile(base, (8, 1))


def _pack_bins(count_vecs, caps_per_bin, nbins, rows_cap=P):
    """Assign rows to nbins bins (<=rows_cap rows each) s.t. per-coordinate
    load sums stay <= caps_per_bin[b].  Returns assignment [n] -> bin, None on
    failure.  caps_per_bin: [nbins, K]."""
    n, k = count_vecs.shape
    caps_per_bin = np.asarray(caps_per_bin, np.int64)
    totals = count_vecs.sum(1)
    order = np.argsort(-totals, kind="stable")
    # deal rows to bins proportionally to bin capacity: snake separately
    # within the heavy prefix and light suffix so the initial load tracks
    # each bin's cap.
    cap_tot = caps_per_bin.sum(1).astype(np.float64)
    share = cap_tot / cap_tot.sum()
    quota = np.round(share * n).astype(np.int64)
    while quota.sum() > n:
        quota[np.argmax(quota)] -= 1
    while quota.sum() < n:
        quota[np.argmin(quota)] += 1
    quota = np.minimum(quota, rows_cap)
    if quota.sum() < n:
        return None
    # snake across bins, skipping bins whose quota is exhausted
    assign = np.empty(n, np.int64)
    fill = np.zeros(nbins, np.int64)
    b = 0
    direction = 1
    for i in range(n):
        while fill[b] >= quota[b]:
            b += direction
            if b == nbins or b < 0:
                direction = -direction
                b += direction
        assign[order[i]] = b
        fill[b] += 1
        b += direction
        if b == nbins or b < 0:
            direction = -direction
            b += direction
    loads = np.zeros((nbins, k), np.int64)
    np.add.at(loads, assign, count_vecs)
    rows = np.bincount(assign, minlength=nbins)
    for _ in range(6000):
        over = loads - caps_per_bin
        bk = np.unravel_index(np.argmax(over), over.shape)
        if over[bk] <= 0:
            return assign
        b, ck = bk
        cand = np.where((assign == b) & (count_vecs[:, ck] > 0))[0]
        cand = cand[np.argsort(count_vecs[cand, ck])]
        slack = caps_per_bin[:, ck] - loads[:, ck]
        tgt_order = np.argsort(-slack, kind="stable")
        moved = False
        for tb in tgt_order:
            if rows[tb] >= rows_cap or tb == b or slack[tb] <= 0:
                continue
            # pick the largest mover that fits everywhere in tb
            for r in cand[::-1]:
                if np.all(loads[tb] + count_vecs[r] <= caps_per_bin[tb]):
                    assign[r] = tb
                    loads[b] -= count_vecs[r]
                    loads[tb] += count_vecs[r]
                    rows[b] -= 1
                    rows[tb] += 1
                    moved = True
                    break
            if moved:
                break
        if not moved:
            return None
    return None


def _bin_node_type(count_mat, caps_per_bin):
    """count_mat [12500, K]; returns (win_of [12500], pos_of [12500],
    wrows [NW,128] slice-local row id or -1)."""
    assign = _pack_bins(count_mat, caps_per_bin, NW)
    if assign is None:
        return None
    win_of = assign
    pos_of = np.empty_like(assign)
    wrows = -np.ones((NW, P), np.int64)
    for w in range(NW):
        rows = np.where(assign == w)[0]
        pos_of[rows] = np.arange(len(rows))
        wrows[w, : len(rows)] = rows
    return win_of, pos_of, wrows


def _edge_meta(src, dst, n_dst, win_of_all, pos_of_all, recip, n_chunks,
               capw):
    """Build per-core gather indices and per-tile metadata for one relation.

    capw: [NW] per-window per-chunk edge capacity (each a multiple of 128).
    Layout: idx16 [C, n_chunks, 128, TOT//16] where TOT = sum(capw); each
    chunk block is the window-major concat of capw[w] segments.
    dl/rc [C, 128, TCOL] where TCOL = n_chunks * sum(capw)//128; col =
    colbase[w] + k*ntile[w] + t, partition = edge position within tile.
    """
    C = NC_CORES
    capw = np.asarray(capw, np.int64)
    ntile_w = capw // P
    TOT = int(capw.sum())
    prefix = np.zeros(NW + 1, np.int64)
    np.cumsum(capw, out=prefix[1:])
    colbase = np.zeros(NW + 1, np.int64)
    np.cumsum(n_chunks * ntile_w, out=colbase[1:])
    TCOL = int(colbase[-1])

    rows_per_core = n_dst // C
    core = dst // rows_per_core
    k = src // S_CHUNK if n_chunks > 1 else np.zeros_like(src)
    w = win_of_all[dst]
    key = (core * NW + w) * n_chunks + k
    order = np.argsort(key, kind="stable")
    key_s = key[order]
    src_s = src[order]
    dst_s = dst[order]
    k_s = k[order]
    w_s = w[order]
    core_s = core[order]
    nseg = C * NW * n_chunks
    seg_counts = np.bincount(key, minlength=nseg)
    segcap = np.tile(np.repeat(capw, n_chunks), C)
    if (seg_counts > segcap).any():
        return None
    seg_start = np.zeros(nseg + 1, np.int64)
    np.cumsum(seg_counts, out=seg_start[1:])
    rank = np.arange(len(src)) - seg_start[key_s]
    # flat edge slot within [C][n_chunks][TOT]
    slot = (core_s * n_chunks + k_s) * TOT + prefix[w_s] + rank
    # flat meta position within [C][TCOL][P]
    mcol = colbase[w_s] + k_s * ntile_w[w_s] + rank // P
    mslot = (core_s * TCOL + mcol) * P + rank % P

    idx_pad = np.zeros(C * n_chunks * TOT, np.int64)
    dl_pad = np.full(C * TCOL * P, -1.0, np.float32)
    rc_pad = np.zeros(C * TCOL * P, np.float32)
    idx_pad[slot] = src_s - k_s * S_CHUNK
    dl_pad[mslot] = pos_of_all[dst_s]
    rc_pad[mslot] = recip[dst_s]

    idx_pad = idx_pad.reshape(C, n_chunks, TOT)
    idx16 = np.empty((C, n_chunks, 128, TOT // 16), np.int16)
    for c in range(C):
        for kk in range(n_chunks):
            idx16[c, kk] = _wrap16(idx_pad[c, kk])
    dl = dl_pad.reshape(C, TCOL, P).transpose(0, 2, 1)
    rc = rc_pad.reshape(C, TCOL, P).transpose(0, 2, 1)
    return np.ascontiguousarray(idx16), np.ascontiguousarray(dl), \
        np.ascontiguousarray(rc)


# ------------------------------------------------------------- device program

def _build_program(ntk4, ntk1, n_user, n_item, n_tag):
    """ntk4: tuple[NW] tiles/chunk for rev & buys; ntk1: tuple[NW] for tags."""
    import concourse.bacc as bacc
    import concourse.bass as bass
    import concourse.mybir as mybir
    from concourse import tile

    f32 = mybir.dt.float32
    f16 = mybir.dt.float16
    i16 = mybir.dt.int16
    TOT4 = sum(ntk4) * P     # edges per chunk block (rev/buys)
    TOT1 = sum(ntk1) * P     # edges per tags block
    TCOL4 = 4 * sum(ntk4)    # meta cols, rev/buys
    TCOL1 = sum(ntk1)

    nc = bacc.Bacc("TRN2", target_bir_lowering=False, debug=False,
                   enable_asserts=False, num_devices=NC_CORES)

    t_xu = nc.dram_tensor("xu", [n_user, P], f16, kind="ExternalInput")
    t_xi = nc.dram_tensor("xi", [n_item, P], f16, kind="ExternalInput")
    t_xt = nc.dram_tensor("xt", [n_tag, P], f16, kind="ExternalInput")
    # host-permuted x_dst tables (window-order rows), per core
    t_xdu = nc.dram_tensor("xdu", [NW * P, P], f16, kind="ExternalInput")
    t_xdi = nc.dram_tensor("xdi", [NW * P, P], f16, kind="ExternalInput")
    # konst: iota | Wl_rev | Wr_rev | b_rev | Wlb | Wlt | Wr_it | ones |
    #        b_u | b_i  (fp16, row0-only for the last three)
    t_const = nc.dram_tensor("konst", [P, 9 * P], f16, kind="ExternalInput")
    t_gi_rev = nc.dram_tensor("gi_rev", [4, 128, TOT4 // 16], i16,
                              kind="ExternalInput")
    t_gi_buys = nc.dram_tensor("gi_buys", [4, 128, TOT4 // 16], i16,
                               kind="ExternalInput")
    t_gi_tags = nc.dram_tensor("gi_tags", [128, TOT1 // 16], i16,
                               kind="ExternalInput")
    t_meta_rev = nc.dram_tensor("meta_rev", [P, 2 * TCOL4], f16,
                                kind="ExternalInput")
    t_meta_buys = nc.dram_tensor("meta_buys", [P, 2 * TCOL4], f16,
                                 kind="ExternalInput")
    t_meta_tags = nc.dram_tensor("meta_tags", [P, 2 * TCOL1], f16,
                                 kind="ExternalInput")
    t_ms_rev = nc.dram_tensor("ms_rev", [P, TCOL4], f32,
                              kind="ExternalInput")
    t_ms_buys = nc.dram_tensor("ms_buys", [P, TCOL4], f32,
                               kind="ExternalInput")
    t_ms_tags = nc.dram_tensor("ms_tags", [P, TCOL1], f32,
                               kind="ExternalInput")
    t_ou = nc.dram_tensor("out_user", [NW * P, P], f32, kind="ExternalOutput")
    t_oi = nc.dram_tensor("out_item", [NW * P, P], f32, kind="ExternalOutput")

    with tile.TileContext(nc) as tc:
        with tc.tile_pool(name="const", bufs=1) as cpool:
            konst = cpool.tile([P, 9 * P], f16)
            nc.sync.dma_start(konst[:], t_const.ap())
            iota = konst[:, 0:P]

            def load_resident(respool, msg_specs, sfx):
                """Load gather-index + meta tiles for a phase up front."""
                prefixes = []
                colbases = []
                for (t_gi, chunks, nch, ntks, t_meta, t_ms) in msg_specs:
                    pr = [0]
                    cb = [0]
                    for w in range(NW):
                        pr.append(pr[-1] + ntks[w] * P)
                        cb.append(cb[-1] + nch * ntks[w])
                    prefixes.append(pr)
                    colbases.append(cb)
                metas = []
                for si, (t_gi, chunks, nch, ntks, t_meta, t_ms) in \
                        enumerate(msg_specs):
                    mt = respool.tile([P, 2 * colbases[si][NW]], f16,
                                      tag=f"meta{sfx}{si}")
                    nc.sync.dma_start(mt[:], t_meta.ap())
                    ms = respool.tile([P, colbases[si][NW]], f32,
                                      tag=f"ms{sfx}{si}")
                    nc.sync.dma_start(ms[:], t_ms.ap())
                    metas.append((mt, ms))
                return prefixes, colbases, metas

            def phase(resident, msg_specs, t_xd, wl_list, wr_col, ones_row,
                      b_row, t_out, pool_sfx, gsizes):
                assert sum(gsizes) == NW
                bounds = [0]
                for gs in gsizes:
                    bounds.append(bounds[-1] + gs)
                GMAXW = max(gsizes)
                """msg_specs: list of (t_gi, gather_chunks_list, n_chunks,
                ntk_list, t_meta)."""
                prefixes, colbases, metas = resident
                with tc.tile_pool(name="gx" + pool_sfx, bufs=1) as gxpool, \
                     tc.tile_pool(name="g" + pool_sfx, bufs=2) as gpool, \
                     tc.tile_pool(name="w" + pool_sfx, bufs=4) as wpool, \
                     tc.tile_pool(name="o" + pool_sfx, bufs=2) as opool, \
                     tc.tile_pool(name="p" + pool_sfx, bufs=2,
                                  space="PSUM") as ppool:
                    gidx_tiles = []
                    for si, (t_gi, chunks, nch, ntks, t_meta, t_ms) in \
                            enumerate(msg_specs):
                        cols = prefixes[si][NW] // 16
                        gt = gxpool.tile([128, nch * cols], i16,
                                         tag=f"gi{pool_sfx}{si}")
                        for kk in range(nch):
                            src_ap = t_gi.ap()[kk] if nch > 1 else t_gi.ap()
                            nc.sync.dma_start(
                                gt[:, kk * cols:(kk + 1) * cols], src_ap)
                        gidx_tiles.append(gt)

                    for g in range(len(gsizes)):
                        g0, g1 = bounds[g], bounds[g + 1]
                        # gathers for this window group
                        xg_bufs = []
                        for si, (t_gi, chunks, nch, ntks, t_meta, t_ms) in \
                                enumerate(msg_specs):
                            cols = prefixes[si][NW] // 16
                            e0, e1 = prefixes[si][g0], prefixes[si][g1]
                            ge = e1 - e0
                            gmax = max(
                                prefixes[si][bounds[a + 1]]
                                - prefixes[si][bounds[a]]
                                for a in range(len(gsizes)))
                            xg = gpool.tile([P, nch * gmax], f16,
                                            tag=f"xg{si}")
                            for kk in range(nch):
                                nc.gpsimd.dma_gather(
                                    out_ap=xg[:, kk * gmax:kk * gmax + ge]
                                    .rearrange("p (t f) -> p t f", f=P),
                                    in_ap=chunks[kk],
                                    idxs_ap=gidx_tiles[si][
                                        :, kk * cols + e0 // 16:
                                        kk * cols + e1 // 16],
                                    num_idxs=ge,
                                    num_idxs_reg=ge,
                                    elem_size=P,
                                    single_packet=False,
                                )
                            xg_bufs.append(xg)
                        # root rows, transposed on the fly: xdT[f, d]
                        xdT = gpool.tile([P, GMAXW * P], f16, tag="xd")
                        nc.sync.dma_start(
                            xdT[:, 0:(g1 - g0) * P],
                            t_xd.ap()[g0 * P:g1 * P, :],
                            transpose=True)

                        for wl_ in range(g1 - g0):
                            w = g0 + wl_
                            msg_sbs = []
                            for si, (t_gi, chunks, nch, ntks, t_meta,
                                     t_ms) in enumerate(msg_specs):
                                ntk = ntks[w]
                                TC = colbases[si][NW]
                                gmax = max(
                                    prefixes[si][bounds[a + 1]]
                                    - prefixes[si][bounds[a]]
                                    for a in range(len(gsizes)))
                                woff = (prefixes[si][w]
                                        - prefixes[si][g0]) // P
                                ps_msg = ppool.tile([P, P], f32,
                                                    space="PSUM",
                                                    tag=f"msg{si}")
                                for kk in range(nch):
                                    for t in range(ntk):
                                        col = (colbases[si][w]
                                               + kk * ntk + t)
                                        oh = wpool.tile([P, P], f16,
                                                        tag=f"oh{si}")
                                        # ACT: oh = relu(rc - rc*(iota-dl)^2)
                                        tmp = wpool.tile(
                                            [P, P], f16, tag=f"tmp{si}")
                                        mt, ms = metas[si]
                                        nc.scalar.activation(
                                            out=tmp[:], in_=iota,
                                            func=mybir.
                                            ActivationFunctionType.Square,
                                            bias=mt[:, col:col + 1],
                                            scale=1.0)
                                        nc.scalar.activation(
                                            out=oh[:], in_=tmp[:],
                                            func=mybir.
                                            ActivationFunctionType.Relu,
                                            bias=mt[:, TC + col:TC + col + 1],
                                            scale=ms[:, col:col + 1])
                                        xg = xg_bufs[si]
                                        tt = kk * (gmax // P) + woff + t
                                        nc.tensor.matmul(
                                            out=ps_msg[:],
                                            lhsT=xg[:, tt * P:(tt + 1) * P],
                                            rhs=oh[:],
                                            start=(kk == 0 and t == 0),
                                            stop=(kk == nch - 1
                                                  and t == ntk - 1),
                                        )
                                msg_sb = wpool.tile([P, P], f16,
                                                    tag=f"msgsb{si}")
                                nc.scalar.copy(out=msg_sb[:], in_=ps_msg[:])
                                msg_sbs.append(msg_sb)

                            ps_out = ppool.tile([P, P], f32, space="PSUM",
                                                tag="out")
                            for si, msg_sb in enumerate(msg_sbs):
                                nc.tensor.matmul(
                                    out=ps_out[:], lhsT=msg_sb[:],
                                    rhs=wl_list[si], start=(si == 0),
                                    stop=False)
                            nc.tensor.matmul(
                                out=ps_out[:],
                                lhsT=xdT[:, wl_ * P:(wl_ + 1) * P],
                                rhs=wr_col, start=False, stop=False)
                            # bias as rank-1: ones(row0) ⊗ b(row0)
                            nc.tensor.matmul(
                                out=ps_out[:], lhsT=ones_row, rhs=b_row,
                                start=False, stop=True)
                            out_sb = opool.tile([P, P], f32, tag="outsb")
                            nc.scalar.copy(out=out_sb[:], in_=ps_out[:])
                            nc.sync.dma_start(
                                t_out.ap()[w * P:(w + 1) * P, :], out_sb[:])

            xi_chunks = [t_xi.ap()[k * S_CHUNK:(k + 1) * S_CHUNK, :]
                         for k in range(4)]
            xu_chunks = [t_xu.ap()[k * S_CHUNK:(k + 1) * S_CHUNK, :]
                         for k in range(4)]
            specs_u = [(t_gi_rev, xi_chunks, 4, ntk4, t_meta_rev,
                        t_ms_rev)]
            specs_i = [
                (t_gi_buys, xu_chunks, 4, ntk4, t_meta_buys, t_ms_buys),
                (t_gi_tags, [t_xt.ap()], 1, ntk1, t_meta_tags, t_ms_tags),
            ]
            with tc.tile_pool(name="res", bufs=1) as respool:
                res_u = load_resident(respool, specs_u, "u")
                res_i = load_resident(respool, specs_i, "i")
                # user phase: relation rev (src=item)
                phase(
                    res_u, msg_specs=specs_u,
                    t_xd=t_xdu,
                    wl_list=[konst[:, 1 * P:2 * P]],
                    wr_col=konst[:, 2 * P:3 * P],
                    ones_row=konst[0:1, 6 * P:7 * P],
                    b_row=konst[0:1, 7 * P:8 * P],
                    t_out=t_ou, pool_sfx="u",
                    gsizes=[20, 20, 20, 20, 10, 5, 5],
                )
                # item phase: relations buys (src=user) + tags (src=tag)
                phase(
                    res_i, msg_specs=specs_i,
                    t_xd=t_xdi,
                    wl_list=[konst[:, 3 * P:4 * P], konst[:, 4 * P:5 * P]],
                    wr_col=konst[:, 5 * P:6 * P],
                    ones_row=konst[0:1, 6 * P:7 * P],
                    b_row=konst[0:1, 8 * P:9 * P],
                    t_out=t_oi, pool_sfx="i",
                    gsizes=[10] * 9 + [5, 5],
                )

    nc.compile()
    return nc


# ------------------------------------------------------------------- kernel()

def kernel(x_user, x_item, x_tag, ei_buys, ei_rev, ei_tags,
           Wl_buys, Wr_buys, b_buys,
           Wl_rev, Wr_rev, b_rev,
           Wl_tags, Wr_tags, b_tags):
    from concourse import bass_utils

    x_user = np.asarray(x_user, np.float32)
    x_item = np.asarray(x_item, np.float32)
    x_tag = np.asarray(x_tag, np.float32)
    xu16 = np.ascontiguousarray(x_user.astype(np.float16))
    xi16 = np.ascontiguousarray(x_item.astype(np.float16))
    xt16 = np.ascontiguousarray(x_tag.astype(np.float16))
    ei_buys = np.asarray(ei_buys, np.int64)
    ei_rev = np.asarray(ei_rev, np.int64)
    ei_tags = np.asarray(ei_tags, np.int64)

    n_user, n_item, n_tag = x_user.shape[0], x_item.shape[0], x_tag.shape[0]
    C = NC_CORES
    ru, ri = n_user // C, n_item // C

    # degree counts + reciprocals per relation (over full dst domain)
    cnt_buys = np.bincount(ei_buys[1], minlength=n_item)
    cnt_rev = np.bincount(ei_rev[1], minlength=n_user)
    cnt_tags = np.bincount(ei_tags[1], minlength=n_item)
    r_buys = (0.5 / np.maximum(cnt_buys, 1)).astype(np.float32)
    r_rev = (1.0 / np.maximum(cnt_rev, 1)).astype(np.float32)
    r_tags = (0.5 / np.maximum(cnt_tags, 1)).astype(np.float32)

    # per-dst-row per-chunk counts for binning
    ch_rev = np.bincount(ei_rev[1] * 4 + ei_rev[0] // S_CHUNK,
                         minlength=n_user * 4).reshape(n_user, 4)
    ch_buys = np.bincount(ei_buys[1] * 4 + ei_buys[0] // S_CHUNK,
                          minlength=n_item * 4).reshape(n_item, 4)

    configs = _CAP_CONFIGS
    m_rev = m_buys = m_tags = None
    for (NH, caph4, capl4, caph1, capl1) in configs:
        NH = min(NH, NW)
        cap4w = np.array([caph4] * NH + [capl4] * (NW - NH), np.int64)
        cap1w = np.array([caph1] * NH + [capl1] * (NW - NH), np.int64)
        ok = True
        win_u = np.empty(n_user, np.int64)
        pos_u = np.empty(n_user, np.int64)
        win_i = np.empty(n_item, np.int64)
        pos_i = np.empty(n_item, np.int64)
        wrows_u = np.empty((C, NW, P), np.int64)
        wrows_i = np.empty((C, NW, P), np.int64)
        caps_u = np.repeat(cap4w[:, None], 4, axis=1)
        caps_i = np.concatenate(
            [np.repeat(cap4w[:, None], 4, axis=1), cap1w[:, None]], axis=1)
        for c in range(C):
            r = _bin_node_type(ch_rev[c * ru:(c + 1) * ru], caps_u)
            if r is None:
                ok = False
                break
            win_u[c * ru:(c + 1) * ru] = r[0]
            pos_u[c * ru:(c + 1) * ru] = r[1]
            wrows_u[c] = r[2]
            cm = np.concatenate(
                [ch_buys[c * ri:(c + 1) * ri],
                 cnt_tags[c * ri:(c + 1) * ri][:, None]], axis=1)
            r = _bin_node_type(cm, caps_i)
            if r is None:
                ok = False
                break
            win_i[c * ri:(c + 1) * ri] = r[0]
            pos_i[c * ri:(c + 1) * ri] = r[1]
            wrows_i[c] = r[2]
        if not ok:
            continue
        m_rev = _edge_meta(ei_rev[0], ei_rev[1], n_user, win_u, pos_u,
                           r_rev, 4, cap4w)
        m_buys = _edge_meta(ei_buys[0], ei_buys[1], n_item, win_i, pos_i,
                            r_buys, 4, cap4w)
        m_tags = _edge_meta(ei_tags[0], ei_tags[1], n_item, win_i, pos_i,
                            r_tags, 1, cap1w)
        if m_rev is not None and m_buys is not None and m_tags is not None:
            break
    assert m_rev is not None and m_buys is not None and m_tags is not None, \
        "binning failed for all capacity configs"
    ntk4 = tuple(int(x) // P for x in cap4w)
    ntk1 = tuple(int(x) // P for x in cap1w)
    gi_rev, dl_rev, rc_rev = m_rev
    gi_buys, dl_buys, rc_buys = m_buys
    gi_tags, dl_tags, rc_tags = m_tags

    # host-permuted x_dst tables: row (w*128+pos) = slice row wrows[w, pos]
    def xd_perm(x16, wrows, c, rows_slice):
        v = wrows[c].reshape(-1).copy()
        v[v < 0] = 0
        return np.ascontiguousarray(
            x16[c * rows_slice + v])

    # constants: iota | Wl_rev | Wr_rev | Wlb | Wlt | Wr_it | ones | b_u | b_i
    iota = np.tile(np.arange(P, dtype=np.float32), (P, 1))
    ones_blk = np.zeros((P, P), np.float32)
    ones_blk[0, :] = 1.0
    bu_blk = np.zeros((P, P), np.float32)
    bu_blk[0, :] = np.asarray(b_rev, np.float32)
    bi_blk = np.zeros((P, P), np.float32)
    bi_blk[0, :] = 0.5 * (np.asarray(b_buys, np.float32)
                          + np.asarray(b_tags, np.float32))
    konst = np.concatenate([
        iota,
        np.asarray(Wl_rev, np.float32), np.asarray(Wr_rev, np.float32),
        np.asarray(Wl_buys, np.float32), np.asarray(Wl_tags, np.float32),
        0.5 * (np.asarray(Wr_buys, np.float32)
               + np.asarray(Wr_tags, np.float32)),
        ones_blk, bu_blk, bi_blk,
    ], axis=1).astype(np.float16)

    key = (ntk4, ntk1, n_user, n_item, n_tag)
    if key not in _COMPILED_CACHE:
        _COMPILED_CACHE[key] = _build_program(*key)
    nc = _COMPILED_CACHE[key]

    def f16m(dl, rc):
        # fp16 planes [-dl | rc]: oh = relu(rc - rc*(iota - dl)^2)
        return np.ascontiguousarray(
            np.concatenate([-dl, rc], axis=1).astype(np.float16))

    def f32s(rc):
        return np.ascontiguousarray((-rc).astype(np.float32))

    in_maps = []
    for c in range(C):
        in_maps.append(dict(
            xu=xu16, xi=xi16, xt=xt16,
            xdu=xd_perm(xu16, wrows_u, c, ru),
            xdi=xd_perm(xi16, wrows_i, c, ri),
            konst=konst,
            gi_rev=gi_rev[c], gi_buys=gi_buys[c], gi_tags=gi_tags[c, 0],
            meta_rev=f16m(dl_rev[c], rc_rev[c]),
            meta_buys=f16m(dl_buys[c], rc_buys[c]),
            meta_tags=f16m(dl_tags[c], rc_tags[c]),
            ms_rev=f32s(rc_rev[c]), ms_buys=f32s(rc_buys[c]),
            ms_tags=f32s(rc_tags[c]),
        ))

    res = bass_utils.run_bass_kernel_spmd(
        nc, in_maps, core_ids=list(range(C)))

    out_user = np.empty((n_user, P), np.float32)
    out_item = np.empty((n_item, P), np.float32)
    for c in range(C):
        ou = res.results[c]["out_user"]
        oi = res.results[c]["out_item"]
        ru_rows = wrows_u[c].reshape(-1)
        ri_rows = wrows_i[c].reshape(-1)
        mu = ru_rows >= 0
        mi = ri_rows >= 0
        out_user[c * ru + ru_rows[mu]] = ou[mu]
        out_item[c * ri + ri_rows[mi]] = oi[mi]
    return out_user, out_item


# revision 14
# speedup vs baseline: 1.1745x; 1.1745x over previous
"""Bass/Trainium2 kernel for a heterogeneous-graph SAGEConv layer (DBGNNLayer).

Strategy: shard by DESTINATION node across the 8 cores (12,500 dst rows of
each node type per core) so no cross-core collectives are needed.  Within a
core, dst rows are packed into 100 windows of 128 rows each, using
load-balanced binning so that every (window, src-chunk) edge segment fits a
fixed capacity (SPMD-uniform static shapes).  Edge source rows are fetched
with dma_gather (int16 indices -> the 100k-row tables are split into 4
chunks of 25k rows) in fp16 (halves HBM gather bytes vs f32).  The
per-window segment mean is computed as a matmul with a scaled one-hot
matrix built on the vector engine in fp16:
    psum_msgT[fin, dstlocal] += Xg_tile[e, fin]^T @ onehot[e, dstlocal]
where onehot[e, d] = (iota[d] == dst_local[e]) * recip[e], recip folding in
the 1/deg mean and the HeteroConv 0.5.  The root term x_dst @ Wr reads a
host-pre-permuted x_dst table with a transposing HWDGE DMA (no gather, no
PE transpose).  The bias is folded into the PSUM accumulation as a rank-1
matmul (ones ⊗ b).  Final per window:
    out[dst, f] = msgT.T @ Wl (+ msgT_tags.T @ Wl_tags) + xdT.T @ Wr + 1⊗b
"""

import sys
import time

sys.path.insert(0, "/opt/trn_rl_repo")

import numpy as np

P = 128                 # partitions / feature dim / window rows
NC_CORES = 8
NW = 100                # windows per node type per core
S_CHUNK = 25000         # rows per gather chunk (int16-safe)
GRP_U = 20              # windows per gather group, user phase
GRP_I = 10              # windows per gather group, item phase

_COMPILED_CACHE = {}

# classed per-window capacities: NH heavy windows, NW-NH light.
# (NH, caph4, capl4, caph1, capl1): rev/buys per-chunk caps; tags caps.
_CAP_CONFIGS = [
    (60, 384, 256, 1408, 1152),       # classed (preferred)
    (100, 384, 384, 1280, 1280),      # uniform fallback
    (100, 512, 512, 1408, 1408),      # enlarged fallback
]


# ----------------------------------------------------------------- host utils

def _wrap16(flat_idx):
    """[n] int -> [128, n//16] int16 wrapped in 16 partitions, replicated."""
    n = flat_idx.shape[0]
    assert n % 16 == 0
    base = flat_idx.reshape(n // 16, 16).T.astype(np.int16)  # [16, n//16]
    return np.tile(base, (8, 1))


def _pack_bins(count_vecs, caps_per_bin, nbins, rows_cap=P):
    """Assign rows to nbins bins (<=rows_cap rows each) s.t. per-coordinate
    load sums stay <= caps_per_bin[b].  Returns assignment [n] -> bin, None on
    failure.  caps_per_bin: [nbins, K]."""
    n, k = count_vecs.shape
    caps_per_bin = np.asarray(caps_per_bin, np.int64)
    totals = count_vecs.sum(1)
    order = np.argsort(-totals, kind="stable")
    # deal rows to bins proportionally to bin capacity: snake separately
    # within the heavy prefix and light suffix so the initial load tracks
    # each bin's cap.
    cap_tot = caps_per_bin.sum(1).astype(np.float64)
    share = cap_tot / cap_tot.sum()
    quota = np.round(share * n).astype(np.int64)
    while quota.sum() > n:
        quota[np.argmax(quota)] -= 1
    while quota.sum() < n:
        quota[np.argmin(quota)] += 1
    quota = np.minimum(quota, rows_cap)
    if quota.sum() < n:
        return None
    # snake across bins, skipping bins whose quota is exhausted
    assign = np.empty(n, np.int64)
    fill = np.zeros(nbins, np.int64)
    b = 0
    direction = 1
    for i in range(n):
        while fill[b] >= quota[b]:
            b += direction
            if b == nbins or b < 0:
                direction = -direction
                b += direction
        assign[order[i]] = b
        fill[b] += 1
        b += direction
        if b == nbins or b < 0:
            direction = -direction
            b += direction
    loads = np.zeros((nbins, k), np.int64)
    np.add.at(loads, assign, count_vecs)
    rows = np.bincount(assign, minlength=nbins)
    for _ in range(6000):
        over = loads - caps_per_bin
        bk = np.unravel_index(np.argmax(over), over.shape)
        if over[bk] <= 0:
            return assign
        b, ck = bk
        cand = np.where((assign == b) & (count_vecs[:, ck] > 0))[0]
        cand = cand[np.argsort(count_vecs[cand, ck])]
        slack = caps_per_bin[:, ck] - loads[:, ck]
        tgt_order = np.argsort(-slack, kind="stable")
        moved = False
        for tb in tgt_order:
            if rows[tb] >= rows_cap or tb == b or slack[tb] <= 0:
                continue
            # pick the largest mover that fits everywhere in tb
            for r in cand[::-1]:
                if np.all(loads[tb] + count_vecs[r] <= caps_per_bin[tb]):
                    assign[r] = tb
                    loads[b] -= count_vecs[r]
                    loads[tb] += count_vecs[r]
                    rows[b] -= 1
                    rows[tb] += 1
                    moved = True
                    break
            if moved:
                break
        if not moved:
            return None
    return None


def _bin_node_type(count_mat, caps_per_bin):
    """count_mat [12500, K]; returns (win_of [12500], pos_of [12500],
    wrows [NW,128] slice-local row id or -1)."""
    assign = _pack_bins(count_mat, caps_per_bin, NW)
    if assign is None:
        return None
    win_of = assign
    pos_of = np.empty_like(assign)
    wrows = -np.ones((NW, P), np.int64)
    for w in range(NW):
        rows = np.where(assign == w)[0]
        pos_of[rows] = np.arange(len(rows))
        wrows[w, : len(rows)] = rows
    return win_of, pos_of, wrows


def _edge_meta(src, dst, n_dst, win_of_all, pos_of_all, recip, n_chunks,
               capw):
    """Build per-core gather indices and per-tile metadata for one relation.

    capw: [NW] per-window per-chunk edge capacity (each a multiple of 128).
    Layout: idx16 [C, n_chunks, 128, TOT//16] where TOT = sum(capw); each
    chunk block is the window-major concat of capw[w] segments.
    dl/rc [C, 128, TCOL] where TCOL = n_chunks * sum(capw)//128; col =
    colbase[w] + k*ntile[w] + t, partition = edge position within tile.
    """
    C = NC_CORES
    capw = np.asarray(capw, np.int64)
    ntile_w = capw // P
    TOT = int(capw.sum())
    prefix = np.zeros(NW + 1, np.int64)
    np.cumsum(capw, out=prefix[1:])
    colbase = np.zeros(NW + 1, np.int64)
    np.cumsum(n_chunks * ntile_w, out=colbase[1:])
    TCOL = int(colbase[-1])

    rows_per_core = n_dst // C
    core = dst // rows_per_core
    k = src // S_CHUNK if n_chunks > 1 else np.zeros_like(src)
    w = win_of_all[dst]
    key = (core * NW + w) * n_chunks + k
    order = np.argsort(key, kind="stable")
    key_s = key[order]
    src_s = src[order]
    dst_s = dst[order]
    k_s = k[order]
    w_s = w[order]
    core_s = core[order]
    nseg = C * NW * n_chunks
    seg_counts = np.bincount(key, minlength=nseg)
    segcap = np.tile(np.repeat(capw, n_chunks), C)
    if (seg_counts > segcap).any():
        return None
    seg_start = np.zeros(nseg + 1, np.int64)
    np.cumsum(seg_counts, out=seg_start[1:])
    rank = np.arange(len(src)) - seg_start[key_s]
    # flat edge slot within [C][n_chunks][TOT]
    slot = (core_s * n_chunks + k_s) * TOT + prefix[w_s] + rank
    # flat meta position within [C][TCOL][P]
    mcol = colbase[w_s] + k_s * ntile_w[w_s] + rank // P
    mslot = (core_s * TCOL + mcol) * P + rank % P

    idx_pad = np.zeros(C * n_chunks * TOT, np.int64)
    dl_pad = np.full(C * TCOL * P, -1.0, np.float32)
    rc_pad = np.zeros(C * TCOL * P, np.float32)
    idx_pad[slot] = src_s - k_s * S_CHUNK
    dl_pad[mslot] = pos_of_all[dst_s]
    rc_pad[mslot] = recip[dst_s]

    idx_pad = idx_pad.reshape(C, n_chunks, TOT)
    idx16 = np.empty((C, n_chunks, 128, TOT // 16), np.int16)
    for c in range(C):
        for kk in range(n_chunks):
            idx16[c, kk] = _wrap16(idx_pad[c, kk])
    dl = dl_pad.reshape(C, TCOL, P).transpose(0, 2, 1)
    rc = rc_pad.reshape(C, TCOL, P).transpose(0, 2, 1)
    return np.ascontiguousarray(idx16), np.ascontiguousarray(dl), \
        np.ascontiguousarray(rc)


# ------------------------------------------------------------- device program

def _build_program(ntk4, ntk1, n_user, n_item, n_tag):
    """ntk4: tuple[NW] tiles/chunk for rev & buys; ntk1: tuple[NW] for tags."""
    import concourse.bacc as bacc
    import concourse.bass as bass
    import concourse.mybir as mybir
    from concourse import tile

    f32 = mybir.dt.float32
    f16 = mybir.dt.float16
    i16 = mybir.dt.int16
    TOT4 = sum(ntk4) * P     # edges per chunk block (rev/buys)
    TOT1 = sum(ntk1) * P     # edges per tags block
    TCOL4 = 4 * sum(ntk4)    # meta cols, rev/buys
    TCOL1 = sum(ntk1)

    nc = bacc.Bacc("TRN2", target_bir_lowering=False, debug=False,
                   enable_asserts=False, num_devices=NC_CORES)

    t_xu = nc.dram_tensor("xu", [n_user, P], f16, kind="ExternalInput")
    t_xi = nc.dram_tensor("xi", [n_item, P], f16, kind="ExternalInput")
    t_xt = nc.dram_tensor("xt", [n_tag, P], f16, kind="ExternalInput")
    # host-permuted x_dst tables (window-order rows), per core
    t_xdu = nc.dram_tensor("xdu", [NW * P, P], f16, kind="ExternalInput")
    t_xdi = nc.dram_tensor("xdi", [NW * P, P], f16, kind="ExternalInput")
    # konst: iota | Wl_rev | Wr_rev | b_rev | Wlb | Wlt | Wr_it | ones |
    #        b_u | b_i  (fp16, row0-only for the last three)
    t_const = nc.dram_tensor("konst", [P, 9 * P], f16, kind="ExternalInput")
    t_gi_rev = nc.dram_tensor("gi_rev", [4, 128, TOT4 // 16], i16,
                              kind="ExternalInput")
    t_gi_buys = nc.dram_tensor("gi_buys", [4, 128, TOT4 // 16], i16,
                               kind="ExternalInput")
    t_gi_tags = nc.dram_tensor("gi_tags", [128, TOT1 // 16], i16,
                               kind="ExternalInput")
    t_meta_rev = nc.dram_tensor("meta_rev", [P, 2 * TCOL4], f16,
                                kind="ExternalInput")
    t_meta_buys = nc.dram_tensor("meta_buys", [P, 2 * TCOL4], f16,
                                 kind="ExternalInput")
    t_meta_tags = nc.dram_tensor("meta_tags", [P, 2 * TCOL1], f16,
                                 kind="ExternalInput")
    t_ms_rev = nc.dram_tensor("ms_rev", [P, TCOL4], f32,
                              kind="ExternalInput")
    t_ms_buys = nc.dram_tensor("ms_buys", [P, TCOL4], f32,
                               kind="ExternalInput")
    t_ms_tags = nc.dram_tensor("ms_tags", [P, TCOL1], f32,
                               kind="ExternalInput")
    t_ou = nc.dram_tensor("out_user", [NW * P, P], f32, kind="ExternalOutput")
    t_oi = nc.dram_tensor("out_item", [NW * P, P], f32, kind="ExternalOutput")

    with tile.TileContext(nc) as tc:
        with tc.tile_pool(name="const", bufs=1) as cpool:
            konst = cpool.tile([P, 9 * P], f16)
            nc.sync.dma_start(konst[:], t_const.ap())
            iota = konst[:, 0:P]

            def load_resident(respool, msg_specs, sfx):
                """Load gather-index + meta tiles for a phase up front."""
                prefixes = []
                colbases = []
                for (t_gi, chunks, nch, ntks, t_meta, t_ms) in msg_specs:
                    pr = [0]
                    cb = [0]
                    for w in range(NW):
                        pr.append(pr[-1] + ntks[w] * P)
                        cb.append(cb[-1] + nch * ntks[w])
                    prefixes.append(pr)
                    colbases.append(cb)
                metas = []
                for si, (t_gi, chunks, nch, ntks, t_meta, t_ms) in \
                        enumerate(msg_specs):
                    mt = respool.tile([P, 2 * colbases[si][NW]], f16,
                                      tag=f"meta{sfx}{si}")
                    nc.sync.dma_start(mt[:], t_meta.ap())
                    ms = respool.tile([P, colbases[si][NW]], f32,
                                      tag=f"ms{sfx}{si}")
                    nc.sync.dma_start(ms[:], t_ms.ap())
                    metas.append((mt, ms))
                return prefixes, colbases, metas

            def phase(resident, msg_specs, t_xd, wl_list, wr_col, ones_row,
                      b_row, t_out, pool_sfx, GRP):
                GRP = min(GRP, NW)
                assert NW % GRP == 0
                bounds = list(range(0, NW + 1, GRP))
                GMAXW = GRP
                gsizes = [GRP] * (NW // GRP)
                """msg_specs: list of (t_gi, gather_chunks_list, n_chunks,
                ntk_list, t_meta)."""
                prefixes, colbases, metas = resident
                with tc.tile_pool(name="gx" + pool_sfx, bufs=1) as gxpool, \
                     tc.tile_pool(name="g" + pool_sfx, bufs=2) as gpool, \
                     tc.tile_pool(name="w" + pool_sfx, bufs=4) as wpool, \
                     tc.tile_pool(name="o" + pool_sfx, bufs=2) as opool, \
                     tc.tile_pool(name="p" + pool_sfx, bufs=2,
                                  space="PSUM") as ppool:
                    gidx_tiles = []
                    for si, (t_gi, chunks, nch, ntks, t_meta, t_ms) in \
                            enumerate(msg_specs):
                        cols = prefixes[si][NW] // 16
                        gt = gxpool.tile([128, nch * cols], i16,
                                         tag=f"gi{pool_sfx}{si}")
                        for kk in range(nch):
                            src_ap = t_gi.ap()[kk] if nch > 1 else t_gi.ap()
                            nc.sync.dma_start(
                                gt[:, kk * cols:(kk + 1) * cols], src_ap)
                        gidx_tiles.append(gt)

                    for g in range(len(gsizes)):
                        g0, g1 = bounds[g], bounds[g + 1]
                        # gathers for this window group
                        xg_bufs = []
                        for si, (t_gi, chunks, nch, ntks, t_meta, t_ms) in \
                                enumerate(msg_specs):
                            cols = prefixes[si][NW] // 16
                            e0, e1 = prefixes[si][g0], prefixes[si][g1]
                            ge = e1 - e0
                            gmax = max(
                                prefixes[si][bounds[a + 1]]
                                - prefixes[si][bounds[a]]
                                for a in range(len(gsizes)))
                            xg = gpool.tile([P, nch * gmax], f16,
                                            tag=f"xg{si}")
                            for kk in range(nch):
                                nc.gpsimd.dma_gather(
                                    out_ap=xg[:, kk * gmax:kk * gmax + ge]
                                    .rearrange("p (t f) -> p t f", f=P),
                                    in_ap=chunks[kk],
                                    idxs_ap=gidx_tiles[si][
                                        :, kk * cols + e0 // 16:
                                        kk * cols + e1 // 16],
                                    num_idxs=ge,
                                    num_idxs_reg=ge,
                                    elem_size=P,
                                    single_packet=False,
                                )
                            xg_bufs.append(xg)
                        # root rows, transposed on the fly: xdT[f, d]
                        xdT = gpool.tile([P, GMAXW * P], f16, tag="xd")
                        nc.sync.dma_start(
                            xdT[:, 0:(g1 - g0) * P],
                            t_xd.ap()[g0 * P:g1 * P, :],
                            transpose=True)

                        for wl_ in range(g1 - g0):
                            w = g0 + wl_
                            msg_sbs = []
                            for si, (t_gi, chunks, nch, ntks, t_meta,
                                     t_ms) in enumerate(msg_specs):
                                ntk = ntks[w]
                                TC = colbases[si][NW]
                                gmax = max(
                                    prefixes[si][bounds[a + 1]]
                                    - prefixes[si][bounds[a]]
                                    for a in range(len(gsizes)))
                                woff = (prefixes[si][w]
                                        - prefixes[si][g0]) // P
                                ps_msg = ppool.tile([P, P], f32,
                                                    space="PSUM",
                                                    tag=f"msg{si}")
                                for kk in range(nch):
                                    for t in range(ntk):
                                        col = (colbases[si][w]
                                               + kk * ntk + t)
                                        oh = wpool.tile([P, P], f16,
                                                        tag=f"oh{si}")
                                        # ACT: oh = relu(rc - rc*(iota-dl)^2)
                                        tmp = wpool.tile(
                                            [P, P], f16, tag=f"tmp{si}")
                                        mt, ms = metas[si]
                                        nc.scalar.activation(
                                            out=tmp[:], in_=iota,
                                            func=mybir.
                                            ActivationFunctionType.Square,
                                            bias=mt[:, col:col + 1],
                                            scale=1.0)
                                        nc.scalar.activation(
                                            out=oh[:], in_=tmp[:],
                                            func=mybir.
                                            ActivationFunctionType.Relu,
                                            bias=mt[:, TC + col:TC + col + 1],
                                            scale=ms[:, col:col + 1])
                                        xg = xg_bufs[si]
                                        tt = kk * (gmax // P) + woff + t
                                        nc.tensor.matmul(
                                            out=ps_msg[:],
                                            lhsT=xg[:, tt * P:(tt + 1) * P],
                                            rhs=oh[:],
                                            start=(kk == 0 and t == 0),
                                            stop=(kk == nch - 1
                                                  and t == ntk - 1),
                                        )
                                msg_sb = wpool.tile([P, P], f16,
                                                    tag=f"msgsb{si}")
                                nc.scalar.copy(out=msg_sb[:], in_=ps_msg[:])
                                msg_sbs.append(msg_sb)

                            ps_out = ppool.tile([P, P], f32, space="PSUM",
                                                tag="out")
                            for si, msg_sb in enumerate(msg_sbs):
                                nc.tensor.matmul(
                                    out=ps_out[:], lhsT=msg_sb[:],
                                    rhs=wl_list[si], start=(si == 0),
                                    stop=False)
                            nc.tensor.matmul(
                                out=ps_out[:],
                                lhsT=xdT[:, wl_ * P:(wl_ + 1) * P],
                                rhs=wr_col, start=False, stop=False)
                            # bias as rank-1: ones(row0) ⊗ b(row0)
                            nc.tensor.matmul(
                                out=ps_out[:], lhsT=ones_row, rhs=b_row,
                                start=False, stop=True)
                            out_sb = opool.tile([P, P], f32, tag="outsb")
                            nc.scalar.copy(out=out_sb[:], in_=ps_out[:])
                            nc.sync.dma_start(
                                t_out.ap()[w * P:(w + 1) * P, :], out_sb[:])

            xi_chunks = [t_xi.ap()[k * S_CHUNK:(k + 1) * S_CHUNK, :]
                         for k in range(4)]
            xu_chunks = [t_xu.ap()[k * S_CHUNK:(k + 1) * S_CHUNK, :]
                         for k in range(4)]
            specs_u = [(t_gi_rev, xi_chunks, 4, ntk4, t_meta_rev,
                        t_ms_rev)]
            specs_i = [
                (t_gi_buys, xu_chunks, 4, ntk4, t_meta_buys, t_ms_buys),
                (t_gi_tags, [t_xt.ap()], 1, ntk1, t_meta_tags, t_ms_tags),
            ]
            with tc.tile_pool(name="res", bufs=1) as respool:
                res_u = load_resident(respool, specs_u, "u")
                res_i = load_resident(respool, specs_i, "i")
                # user phase: relation rev (src=item)
                phase(
                    res_u, msg_specs=specs_u,
                    t_xd=t_xdu,
                    wl_list=[konst[:, 1 * P:2 * P]],
                    wr_col=konst[:, 2 * P:3 * P],
                    ones_row=konst[0:1, 6 * P:7 * P],
                    b_row=konst[0:1, 7 * P:8 * P],
                    t_out=t_ou, pool_sfx="u", GRP=GRP_U,
                )
                # item phase: relations buys (src=user) + tags (src=tag)
                phase(
                    res_i, msg_specs=specs_i,
                    t_xd=t_xdi,
                    wl_list=[konst[:, 3 * P:4 * P], konst[:, 4 * P:5 * P]],
                    wr_col=konst[:, 5 * P:6 * P],
                    ones_row=konst[0:1, 6 * P:7 * P],
                    b_row=konst[0:1, 8 * P:9 * P],
                    t_out=t_oi, pool_sfx="i", GRP=GRP_I,
                )

    nc.compile()
    return nc


# ------------------------------------------------------------------- kernel()

def kernel(x_user, x_item, x_tag, ei_buys, ei_rev, ei_tags,
           Wl_buys, Wr_buys, b_buys,
           Wl_rev, Wr_rev, b_rev,
           Wl_tags, Wr_tags, b_tags):
    from concourse import bass_utils

    x_user = np.asarray(x_user, np.float32)
    x_item = np.asarray(x_item, np.float32)
    x_tag = np.asarray(x_tag, np.float32)
    xu16 = np.ascontiguousarray(x_user.astype(np.float16))
    xi16 = np.ascontiguousarray(x_item.astype(np.float16))
    xt16 = np.ascontiguousarray(x_tag.astype(np.float16))
    ei_buys = np.asarray(ei_buys, np.int64)
    ei_rev = np.asarray(ei_rev, np.int64)
    ei_tags = np.asarray(ei_tags, np.int64)

    n_user, n_item, n_tag = x_user.shape[0], x_item.shape[0], x_tag.shape[0]
    C = NC_CORES
    ru, ri = n_user // C, n_item // C

    # degree counts + reciprocals per relation (over full dst domain)
    cnt_buys = np.bincount(ei_buys[1], minlength=n_item)
    cnt_rev = np.bincount(ei_rev[1], minlength=n_user)
    cnt_tags = np.bincount(ei_tags[1], minlength=n_item)
    r_buys = (0.5 / np.maximum(cnt_buys, 1)).astype(np.float32)
    r_rev = (1.0 / np.maximum(cnt_rev, 1)).astype(np.float32)
    r_tags = (0.5 / np.maximum(cnt_tags, 1)).astype(np.float32)

    # per-dst-row per-chunk counts for binning
    ch_rev = np.bincount(ei_rev[1] * 4 + ei_rev[0] // S_CHUNK,
                         minlength=n_user * 4).reshape(n_user, 4)
    ch_buys = np.bincount(ei_buys[1] * 4 + ei_buys[0] // S_CHUNK,
                          minlength=n_item * 4).reshape(n_item, 4)

    configs = _CAP_CONFIGS
    m_rev = m_buys = m_tags = None
    for (NH, caph4, capl4, caph1, capl1) in configs:
        NH = min(NH, NW)
        cap4w = np.array([caph4] * NH + [capl4] * (NW - NH), np.int64)
        cap1w = np.array([caph1] * NH + [capl1] * (NW - NH), np.int64)
        ok = True
        win_u = np.empty(n_user, np.int64)
        pos_u = np.empty(n_user, np.int64)
        win_i = np.empty(n_item, np.int64)
        pos_i = np.empty(n_item, np.int64)
        wrows_u = np.empty((C, NW, P), np.int64)
        wrows_i = np.empty((C, NW, P), np.int64)
        caps_u = np.repeat(cap4w[:, None], 4, axis=1)
        caps_i = np.concatenate(
            [np.repeat(cap4w[:, None], 4, axis=1), cap1w[:, None]], axis=1)
        for c in range(C):
            r = _bin_node_type(ch_rev[c * ru:(c + 1) * ru], caps_u)
            if r is None:
                ok = False
                break
            win_u[c * ru:(c + 1) * ru] = r[0]
            pos_u[c * ru:(c + 1) * ru] = r[1]
            wrows_u[c] = r[2]
            cm = np.concatenate(
                [ch_buys[c * ri:(c + 1) * ri],
                 cnt_tags[c * ri:(c + 1) * ri][:, None]], axis=1)
            r = _bin_node_type(cm, caps_i)
            if r is None:
                ok = False
                break
            win_i[c * ri:(c + 1) * ri] = r[0]
            pos_i[c * ri:(c + 1) * ri] = r[1]
            wrows_i[c] = r[2]
        if not ok:
            continue
        m_rev = _edge_meta(ei_rev[0], ei_rev[1], n_user, win_u, pos_u,
                           r_rev, 4, cap4w)
        m_buys = _edge_meta(ei_buys[0], ei_buys[1], n_item, win_i, pos_i,
                            r_buys, 4, cap4w)
        m_tags = _edge_meta(ei_tags[0], ei_tags[1], n_item, win_i, pos_i,
                            r_tags, 1, cap1w)
        if m_rev is not None and m_buys is not None and m_tags is not None:
            break
    assert m_rev is not None and m_buys is not None and m_tags is not None, \
        "binning failed for all capacity configs"
    ntk4 = tuple(int(x) // P for x in cap4w)
    ntk1 = tuple(int(x) // P for x in cap1w)
    gi_rev, dl_rev, rc_rev = m_rev
    gi_buys, dl_buys, rc_buys = m_buys
    gi_tags, dl_tags, rc_tags = m_tags

    # host-permuted x_dst tables: row (w*128+pos) = slice row wrows[w, pos]
    def xd_perm(x16, wrows, c, rows_slice):
        v = wrows[c].reshape(-1).copy()
        v[v < 0] = 0
        return np.ascontiguousarray(
            x16[c * rows_slice + v])

    # constants: iota | Wl_rev | Wr_rev | Wlb | Wlt | Wr_it | ones | b_u | b_i
    iota = np.tile(np.arange(P, dtype=np.float32), (P, 1))
    ones_blk = np.zeros((P, P), np.float32)
    ones_blk[0, :] = 1.0
    bu_blk = np.zeros((P, P), np.float32)
    bu_blk[0, :] = np.asarray(b_rev, np.float32)
    bi_blk = np.zeros((P, P), np.float32)
    bi_blk[0, :] = 0.5 * (np.asarray(b_buys, np.float32)
                          + np.asarray(b_tags, np.float32))
    konst = np.concatenate([
        iota,
        np.asarray(Wl_rev, np.float32), np.asarray(Wr_rev, np.float32),
        np.asarray(Wl_buys, np.float32), np.asarray(Wl_tags, np.float32),
        0.5 * (np.asarray(Wr_buys, np.float32)
               + np.asarray(Wr_tags, np.float32)),
        ones_blk, bu_blk, bi_blk,
    ], axis=1).astype(np.float16)

    key = (ntk4, ntk1, n_user, n_item, n_tag)
    if key not in _COMPILED_CACHE:
        _COMPILED_CACHE[key] = _build_program(*key)
    nc = _COMPILED_CACHE[key]

    def f16m(dl, rc):
        # fp16 planes [-dl | rc]: oh = relu(rc - rc*(iota - dl)^2)
        return np.ascontiguousarray(
            np.concatenate([-dl, rc], axis=1).astype(np.float16))

    def f32s(rc):
        return np.ascontiguousarray((-rc).astype(np.float32))

    in_maps = []
    for c in range(C):
        in_maps.append(dict(
            xu=xu16, xi=xi16, xt=xt16,
            xdu=xd_perm(xu16, wrows_u, c, ru),
            xdi=xd_perm(xi16, wrows_i, c, ri),
            konst=konst,
            gi_rev=gi_rev[c], gi_buys=gi_buys[c], gi_tags=gi_tags[c, 0],
            meta_rev=f16m(dl_rev[c], rc_rev[c]),
            meta_buys=f16m(dl_buys[c], rc_buys[c]),
            meta_tags=f16m(dl_tags[c], rc_tags[c]),
            ms_rev=f32s(rc_rev[c]), ms_buys=f32s(rc_buys[c]),
            ms_tags=f32s(rc_tags[c]),
        ))

    res = bass_utils.run_bass_kernel_spmd(
        nc, in_maps, core_ids=list(range(C)))

    out_user = np.empty((n_user, P), np.float32)
    out_item = np.empty((n_item, P), np.float32)
    for c in range(C):
        ou = res.results[c]["out_user"]
        oi = res.results[c]["out_item"]
        ru_rows = wrows_u[c].reshape(-1)
        ri_rows = wrows_i[c].reshape(-1)
        mu = ru_rows >= 0
        mi = ri_rows >= 0
        out_user[c * ru + ru_rows[mu]] = ou[mu]
        out_item[c * ri + ri_rows[mi]] = oi[mi]
    return out_user, out_item


# revision 17
# speedup vs baseline: 1.2857x; 1.0947x over previous
"""Bass/Trainium2 kernel for a heterogeneous-graph SAGEConv layer (DBGNNLayer).

Strategy: shard by DESTINATION node across the 8 cores (12,500 dst rows of
each node type per core) so no cross-core collectives are needed.  Within a
core, dst rows are packed into 100 windows of 128 rows each, using
load-balanced binning so that every (window, src-chunk) edge segment fits a
fixed capacity (SPMD-uniform static shapes).  Edge source rows are fetched
with dma_gather (int16 indices -> the 100k-row tables are split into 4
chunks of 25k rows) in fp16 (halves HBM gather bytes vs f32).  The
per-window segment mean is computed as a matmul with a scaled one-hot
matrix built on the vector engine in fp16:
    psum_msgT[fin, dstlocal] += Xg_tile[e, fin]^T @ onehot[e, dstlocal]
where onehot[e, d] = (iota[d] == dst_local[e]) * recip[e], recip folding in
the 1/deg mean and the HeteroConv 0.5.  The root term x_dst @ Wr reads a
host-pre-permuted x_dst table with a transposing HWDGE DMA (no gather, no
PE transpose).  The bias is folded into the PSUM accumulation as a rank-1
matmul (ones ⊗ b).  Final per window:
    out[dst, f] = msgT.T @ Wl (+ msgT_tags.T @ Wl_tags) + xdT.T @ Wr + 1⊗b
"""

import sys
import time

sys.path.insert(0, "/opt/trn_rl_repo")

import numpy as np

P = 128                 # partitions / feature dim / window rows
NC_CORES = 8
NW = 100                # windows per node type per core
S_CHUNK = 25000         # rows per gather chunk (int16-safe)
GRP_U = 20              # windows per gather group, user phase
GRP_I = 10              # windows per gather group, item phase

_COMPILED_CACHE = {}

# classed per-window capacities: NH heavy windows, NW-NH light.
# (NH, caph4, capl4, caph1, capl1): rev/buys per-chunk caps; tags caps.
# (NH_user, NH_item, caph4, capl4, caph1, capl1)
_CAP_CONFIGS = [
    (60, 70, 384, 256, 1408, 1152),   # classed (preferred)
    (70, 80, 384, 256, 1408, 1152),   # looser classed
    (100, 100, 384, 384, 1280, 1280),  # uniform fallback
    (100, 100, 512, 512, 1408, 1408),  # enlarged fallback
]


# ----------------------------------------------------------------- host utils

def _wrap16(flat_idx):
    """[n] int -> [128, n//16] int16 wrapped in 16 partitions, replicated."""
    n = flat_idx.shape[0]
    assert n % 16 == 0
    base = flat_idx.reshape(n // 16, 16).T.astype(np.int16)  # [16, n//16]
    return np.tile(base, (8, 1))


def _pack_bins(count_vecs, caps_per_bin, nbins, rows_cap=P):
    """Assign rows to nbins bins (<=rows_cap rows each) s.t. per-coordinate
    load sums stay <= caps_per_bin[b].  Returns assignment [n] -> bin, None on
    failure.  caps_per_bin: [nbins, K]."""
    n, k = count_vecs.shape
    caps_per_bin = np.asarray(caps_per_bin, np.int64)
    totals = count_vecs.sum(1)
    order = np.argsort(-totals, kind="stable")
    # deal rows to bins proportionally to bin capacity: snake separately
    # within the heavy prefix and light suffix so the initial load tracks
    # each bin's cap.
    cap_tot = caps_per_bin.sum(1).astype(np.float64)
    share = cap_tot / cap_tot.sum()
    quota = np.round(share * n).astype(np.int64)
    while quota.sum() > n:
        quota[np.argmax(quota)] -= 1
    while quota.sum() < n:
        quota[np.argmin(quota)] += 1
    quota = np.minimum(quota, rows_cap)
    if quota.sum() < n:
        return None
    # snake across bins, skipping bins whose quota is exhausted
    assign = np.empty(n, np.int64)
    fill = np.zeros(nbins, np.int64)
    b = 0
    direction = 1
    for i in range(n):
        while fill[b] >= quota[b]:
            b += direction
            if b == nbins or b < 0:
                direction = -direction
                b += direction
        assign[order[i]] = b
        fill[b] += 1
        b += direction
        if b == nbins or b < 0:
            direction = -direction
            b += direction
    loads = np.zeros((nbins, k), np.int64)
    np.add.at(loads, assign, count_vecs)
    rows = np.bincount(assign, minlength=nbins)
    for _ in range(6000):
        over = loads - caps_per_bin
        bk = np.unravel_index(np.argmax(over), over.shape)
        if over[bk] <= 0:
            return assign
        b, ck = bk
        cand = np.where((assign == b) & (count_vecs[:, ck] > 0))[0]
        cand = cand[np.argsort(count_vecs[cand, ck])]
        slack = caps_per_bin[:, ck] - loads[:, ck]
        tgt_order = np.argsort(-slack, kind="stable")
        moved = False
        for tb in tgt_order:
            if rows[tb] >= rows_cap or tb == b or slack[tb] <= 0:
                continue
            # pick the largest mover that fits everywhere in tb
            for r in cand[::-1]:
                if np.all(loads[tb] + count_vecs[r] <= caps_per_bin[tb]):
                    assign[r] = tb
                    loads[b] -= count_vecs[r]
                    loads[tb] += count_vecs[r]
                    rows[b] -= 1
                    rows[tb] += 1
                    moved = True
                    break
            if moved:
                break
        if not moved:
            return None
    return None


def _pack_classed(count_vecs, nh, cap_h, cap_l, rows_cap=P):
    """Two-stage packer: split rows into heavy/light classes by total load,
    then LPT-balance each class across its uniform-cap bins."""
    n, k = count_vecs.shape
    nl = NW - nh
    cap_h = np.asarray(cap_h, np.float64)
    cap_l = np.asarray(cap_l, np.float64)
    order = np.argsort(-count_vecs.sum(1), kind="stable")
    split = None
    for q_l in (nl * rows_cap, int(nl * rows_cap * 0.98),
                int(nl * rows_cap * 0.95)):
        q_l = min(q_l, n)
        q_h = n - q_l
        if q_h > nh * rows_cap:
            continue
        if (count_vecs[order[q_h:]].sum(0) > cap_l * nl).any():
            continue
        if (count_vecs[order[:q_h]].sum(0) > cap_h * nh).any():
            continue
        split = q_h
        break
    if split is None:
        return None
    assign = np.empty(n, np.int64)

    def lpt(rows_idx, nbins, caps, bin0):
        loads = np.zeros((nbins, k))
        cnt = np.zeros(nbins, np.int64)
        capsafe = np.maximum(caps, 1)
        for i in rows_idx:
            v = count_vecs[i]
            fits = (cnt < rows_cap) & np.all(loads + v <= caps, axis=1)
            if not fits.any():
                return False
            rem = (caps - loads - v) / capsafe
            score = rem.min(axis=1) + 1e-4 * (rows_cap - cnt)
            score[~fits] = -np.inf
            b = int(np.argmax(score))
            assign[i] = bin0 + b
            loads[b] += v
            cnt[b] += 1
        return True

    if not lpt(order[:split], nh, cap_h, 0):
        return None
    if not lpt(order[split:], nl, cap_l, nh):
        return None
    return assign


def _bin_node_type(count_mat, caps_per_bin):
    """count_mat [12500, K]; returns (win_of [12500], pos_of [12500],
    wrows [NW,128] slice-local row id or -1)."""
    caps_arr = np.asarray(caps_per_bin)
    nh = int((caps_arr[:, 0] == caps_arr[0, 0]).sum()) \
        if (caps_arr[0, 0] != caps_arr[-1, 0]) else NW
    if nh < NW:
        assign = _pack_classed(count_mat, nh, caps_arr[0], caps_arr[-1])
    else:
        assign = None
    if assign is None:
        assign = _pack_bins(count_mat, caps_per_bin, NW)
    if assign is None:
        return None
    win_of = assign
    pos_of = np.empty_like(assign)
    wrows = -np.ones((NW, P), np.int64)
    for w in range(NW):
        rows = np.where(assign == w)[0]
        pos_of[rows] = np.arange(len(rows))
        wrows[w, : len(rows)] = rows
    return win_of, pos_of, wrows


def _edge_meta(src, dst, n_dst, win_of_all, pos_of_all, recip, n_chunks,
               capw):
    """Build per-core gather indices and per-tile metadata for one relation.

    capw: [NW] per-window per-chunk edge capacity (each a multiple of 128).
    Layout: idx16 [C, n_chunks, 128, TOT//16] where TOT = sum(capw); each
    chunk block is the window-major concat of capw[w] segments.
    dl/rc [C, 128, TCOL] where TCOL = n_chunks * sum(capw)//128; col =
    colbase[w] + k*ntile[w] + t, partition = edge position within tile.
    """
    C = NC_CORES
    capw = np.asarray(capw, np.int64)
    ntile_w = capw // P
    TOT = int(capw.sum())
    prefix = np.zeros(NW + 1, np.int64)
    np.cumsum(capw, out=prefix[1:])
    colbase = np.zeros(NW + 1, np.int64)
    np.cumsum(n_chunks * ntile_w, out=colbase[1:])
    TCOL = int(colbase[-1])

    rows_per_core = n_dst // C
    core = dst // rows_per_core
    k = src // S_CHUNK if n_chunks > 1 else np.zeros_like(src)
    w = win_of_all[dst]
    key = (core * NW + w) * n_chunks + k
    order = np.argsort(key, kind="stable")
    key_s = key[order]
    src_s = src[order]
    dst_s = dst[order]
    k_s = k[order]
    w_s = w[order]
    core_s = core[order]
    nseg = C * NW * n_chunks
    seg_counts = np.bincount(key, minlength=nseg)
    segcap = np.tile(np.repeat(capw, n_chunks), C)
    if (seg_counts > segcap).any():
        return None
    seg_start = np.zeros(nseg + 1, np.int64)
    np.cumsum(seg_counts, out=seg_start[1:])
    rank = np.arange(len(src)) - seg_start[key_s]
    # flat edge slot within [C][n_chunks][TOT]
    slot = (core_s * n_chunks + k_s) * TOT + prefix[w_s] + rank
    # flat meta position within [C][TCOL][P]
    mcol = colbase[w_s] + k_s * ntile_w[w_s] + rank // P
    mslot = (core_s * TCOL + mcol) * P + rank % P

    idx_pad = np.zeros(C * n_chunks * TOT, np.int64)
    dl_pad = np.full(C * TCOL * P, -1.0, np.float32)
    rc_pad = np.zeros(C * TCOL * P, np.float32)
    idx_pad[slot] = src_s - k_s * S_CHUNK
    dl_pad[mslot] = pos_of_all[dst_s]
    rc_pad[mslot] = recip[dst_s]

    idx_pad = idx_pad.reshape(C, n_chunks, TOT)
    idx16 = np.empty((C, n_chunks, 128, TOT // 16), np.int16)
    for c in range(C):
        for kk in range(n_chunks):
            idx16[c, kk] = _wrap16(idx_pad[c, kk])
    dl = dl_pad.reshape(C, TCOL, P).transpose(0, 2, 1)
    rc = rc_pad.reshape(C, TCOL, P).transpose(0, 2, 1)
    return np.ascontiguousarray(idx16), np.ascontiguousarray(dl), \
        np.ascontiguousarray(rc)


# ------------------------------------------------------------- device program

def _build_program(ntk_rev, ntk_buys, ntk1, n_user, n_item, n_tag):
    """ntk_*: tuple[NW] tiles/chunk per relation."""
    import concourse.bacc as bacc
    import concourse.bass as bass
    import concourse.mybir as mybir
    from concourse import tile

    f32 = mybir.dt.float32
    f16 = mybir.dt.float16
    i16 = mybir.dt.int16
    TOTR = sum(ntk_rev) * P
    TOTB = sum(ntk_buys) * P
    TOT1 = sum(ntk1) * P     # edges per tags block
    TCOLR = 4 * sum(ntk_rev)
    TCOLB = 4 * sum(ntk_buys)
    TCOL1 = sum(ntk1)

    nc = bacc.Bacc("TRN2", target_bir_lowering=False, debug=False,
                   enable_asserts=False, num_devices=NC_CORES)

    t_xu = nc.dram_tensor("xu", [n_user, P], f16, kind="ExternalInput")
    t_xi = nc.dram_tensor("xi", [n_item, P], f16, kind="ExternalInput")
    t_xt = nc.dram_tensor("xt", [n_tag, P], f16, kind="ExternalInput")
    # host-permuted x_dst tables (window-order rows), per core
    t_xdu = nc.dram_tensor("xdu", [NW * P, P], f16, kind="ExternalInput")
    t_xdi = nc.dram_tensor("xdi", [NW * P, P], f16, kind="ExternalInput")
    # konst: iota | Wl_rev | Wr_rev | b_rev | Wlb | Wlt | Wr_it | ones |
    #        b_u | b_i  (fp16, row0-only for the last three)
    t_const = nc.dram_tensor("konst", [P, 9 * P], f16, kind="ExternalInput")
    t_gi_rev = nc.dram_tensor("gi_rev", [4, 128, TOTR // 16], i16,
                              kind="ExternalInput")
    t_gi_buys = nc.dram_tensor("gi_buys", [4, 128, TOTB // 16], i16,
                               kind="ExternalInput")
    t_gi_tags = nc.dram_tensor("gi_tags", [128, TOT1 // 16], i16,
                               kind="ExternalInput")
    t_meta_rev = nc.dram_tensor("meta_rev", [P, 2 * TCOLR], f16,
                                kind="ExternalInput")
    t_meta_buys = nc.dram_tensor("meta_buys", [P, 2 * TCOLB], f16,
                                 kind="ExternalInput")
    t_meta_tags = nc.dram_tensor("meta_tags", [P, 2 * TCOL1], f16,
                                 kind="ExternalInput")
    t_ms_rev = nc.dram_tensor("ms_rev", [P, TCOLR], f32,
                              kind="ExternalInput")
    t_ms_buys = nc.dram_tensor("ms_buys", [P, TCOLB], f32,
                               kind="ExternalInput")
    t_ms_tags = nc.dram_tensor("ms_tags", [P, TCOL1], f32,
                               kind="ExternalInput")
    t_ou = nc.dram_tensor("out_user", [NW * P, P], f32, kind="ExternalOutput")
    t_oi = nc.dram_tensor("out_item", [NW * P, P], f32, kind="ExternalOutput")

    with tile.TileContext(nc) as tc:
        with tc.tile_pool(name="const", bufs=1) as cpool:
            konst = cpool.tile([P, 9 * P], f16)
            nc.sync.dma_start(konst[:], t_const.ap())
            iota = konst[:, 0:P]

            def load_resident(respool, msg_specs, sfx):
                """Load gather-index + meta tiles for a phase up front."""
                prefixes = []
                colbases = []
                for (t_gi, chunks, nch, ntks, t_meta, t_ms) in msg_specs:
                    pr = [0]
                    cb = [0]
                    for w in range(NW):
                        pr.append(pr[-1] + ntks[w] * P)
                        cb.append(cb[-1] + nch * ntks[w])
                    prefixes.append(pr)
                    colbases.append(cb)
                metas = []
                for si, (t_gi, chunks, nch, ntks, t_meta, t_ms) in \
                        enumerate(msg_specs):
                    mt = respool.tile([P, 2 * colbases[si][NW]], f16,
                                      tag=f"meta{sfx}{si}")
                    nc.sync.dma_start(mt[:], t_meta.ap())
                    ms = respool.tile([P, colbases[si][NW]], f32,
                                      tag=f"ms{sfx}{si}")
                    nc.sync.dma_start(ms[:], t_ms.ap())
                    metas.append((mt, ms))
                return prefixes, colbases, metas

            def phase(resident, msg_specs, t_xd, wl_list, wr_col, ones_row,
                      b_row, t_out, pool_sfx, GRP):
                GRP = min(GRP, NW)
                assert NW % GRP == 0
                bounds = list(range(0, NW + 1, GRP))
                GMAXW = GRP
                gsizes = [GRP] * (NW // GRP)
                """msg_specs: list of (t_gi, gather_chunks_list, n_chunks,
                ntk_list, t_meta)."""
                prefixes, colbases, metas = resident
                with tc.tile_pool(name="gx" + pool_sfx, bufs=1) as gxpool, \
                     tc.tile_pool(name="g" + pool_sfx, bufs=2) as gpool, \
                     tc.tile_pool(name="w" + pool_sfx, bufs=4) as wpool, \
                     tc.tile_pool(name="o" + pool_sfx, bufs=2) as opool, \
                     tc.tile_pool(name="p" + pool_sfx, bufs=2,
                                  space="PSUM") as ppool:
                    gidx_tiles = []
                    for si, (t_gi, chunks, nch, ntks, t_meta, t_ms) in \
                            enumerate(msg_specs):
                        cols = prefixes[si][NW] // 16
                        gt = gxpool.tile([128, nch * cols], i16,
                                         tag=f"gi{pool_sfx}{si}")
                        for kk in range(nch):
                            src_ap = t_gi.ap()[kk] if nch > 1 else t_gi.ap()
                            nc.sync.dma_start(
                                gt[:, kk * cols:(kk + 1) * cols], src_ap)
                        gidx_tiles.append(gt)

                    for g in range(len(gsizes)):
                        g0, g1 = bounds[g], bounds[g + 1]
                        # gathers for this window group
                        xg_bufs = []
                        for si, (t_gi, chunks, nch, ntks, t_meta, t_ms) in \
                                enumerate(msg_specs):
                            cols = prefixes[si][NW] // 16
                            e0, e1 = prefixes[si][g0], prefixes[si][g1]
                            ge = e1 - e0
                            gmax = max(
                                prefixes[si][bounds[a + 1]]
                                - prefixes[si][bounds[a]]
                                for a in range(len(gsizes)))
                            xg = gpool.tile([P, nch * gmax], f16,
                                            tag=f"xg{si}")
                            for kk in range(nch):
                                nc.gpsimd.dma_gather(
                                    out_ap=xg[:, kk * gmax:kk * gmax + ge]
                                    .rearrange("p (t f) -> p t f", f=P),
                                    in_ap=chunks[kk],
                                    idxs_ap=gidx_tiles[si][
                                        :, kk * cols + e0 // 16:
                                        kk * cols + e1 // 16],
                                    num_idxs=ge,
                                    num_idxs_reg=ge,
                                    elem_size=P,
                                    single_packet=False,
                                )
                            xg_bufs.append(xg)
                        # root rows, transposed on the fly: xdT[f, d]
                        xdT = gpool.tile([P, GMAXW * P], f16, tag="xd")
                        nc.sync.dma_start(
                            xdT[:, 0:(g1 - g0) * P],
                            t_xd.ap()[g0 * P:g1 * P, :],
                            transpose=True)

                        for wl_ in range(g1 - g0):
                            w = g0 + wl_
                            msg_sbs = []
                            for si, (t_gi, chunks, nch, ntks, t_meta,
                                     t_ms) in enumerate(msg_specs):
                                ntk = ntks[w]
                                TC = colbases[si][NW]
                                gmax = max(
                                    prefixes[si][bounds[a + 1]]
                                    - prefixes[si][bounds[a]]
                                    for a in range(len(gsizes)))
                                woff = (prefixes[si][w]
                                        - prefixes[si][g0]) // P
                                ps_msg = ppool.tile([P, P], f32,
                                                    space="PSUM",
                                                    tag=f"msg{si}")
                                for kk in range(nch):
                                    for t in range(ntk):
                                        col = (colbases[si][w]
                                               + kk * ntk + t)
                                        oh = wpool.tile([P, P], f16,
                                                        tag=f"oh{si}")
                                        # ACT: oh = relu(rc - rc*(iota-dl)^2)
                                        tmp = wpool.tile(
                                            [P, P], f16, tag=f"tmp{si}")
                                        mt, ms = metas[si]
                                        nc.scalar.activation(
                                            out=tmp[:], in_=iota,
                                            func=mybir.
                                            ActivationFunctionType.Square,
                                            bias=mt[:, col:col + 1],
                                            scale=1.0)
                                        nc.scalar.activation(
                                            out=oh[:], in_=tmp[:],
                                            func=mybir.
                                            ActivationFunctionType.Relu,
                                            bias=mt[:, TC + col:TC + col + 1],
                                            scale=ms[:, col:col + 1])
                                        xg = xg_bufs[si]
                                        tt = kk * (gmax // P) + woff + t
                                        nc.tensor.matmul(
                                            out=ps_msg[:],
                                            lhsT=xg[:, tt * P:(tt + 1) * P],
                                            rhs=oh[:],
                                            start=(kk == 0 and t == 0),
                                            stop=(kk == nch - 1
                                                  and t == ntk - 1),
                                        )
                                msg_sb = wpool.tile([P, P], f16,
                                                    tag=f"msgsb{si}")
                                nc.scalar.copy(out=msg_sb[:], in_=ps_msg[:])
                                msg_sbs.append(msg_sb)

                            ps_out = ppool.tile([P, P], f32, space="PSUM",
                                                tag="out")
                            for si, msg_sb in enumerate(msg_sbs):
                                nc.tensor.matmul(
                                    out=ps_out[:], lhsT=msg_sb[:],
                                    rhs=wl_list[si], start=(si == 0),
                                    stop=False)
                            nc.tensor.matmul(
                                out=ps_out[:],
                                lhsT=xdT[:, wl_ * P:(wl_ + 1) * P],
                                rhs=wr_col, start=False, stop=False)
                            # bias as rank-1: ones(row0) ⊗ b(row0)
                            nc.tensor.matmul(
                                out=ps_out[:], lhsT=ones_row, rhs=b_row,
                                start=False, stop=True)
                            out_sb = opool.tile([P, P], f32, tag="outsb")
                            nc.scalar.copy(out=out_sb[:], in_=ps_out[:])
                            nc.sync.dma_start(
                                t_out.ap()[w * P:(w + 1) * P, :], out_sb[:])

            xi_chunks = [t_xi.ap()[k * S_CHUNK:(k + 1) * S_CHUNK, :]
                         for k in range(4)]
            xu_chunks = [t_xu.ap()[k * S_CHUNK:(k + 1) * S_CHUNK, :]
                         for k in range(4)]
            specs_u = [(t_gi_rev, xi_chunks, 4, ntk_rev, t_meta_rev,
                        t_ms_rev)]
            specs_i = [
                (t_gi_buys, xu_chunks, 4, ntk_buys, t_meta_buys, t_ms_buys),
                (t_gi_tags, [t_xt.ap()], 1, ntk1, t_meta_tags, t_ms_tags),
            ]
            with tc.tile_pool(name="res", bufs=1) as respool:
                res_u = load_resident(respool, specs_u, "u")
                res_i = load_resident(respool, specs_i, "i")
                # user phase: relation rev (src=item)
                phase(
                    res_u, msg_specs=specs_u,
                    t_xd=t_xdu,
                    wl_list=[konst[:, 1 * P:2 * P]],
                    wr_col=konst[:, 2 * P:3 * P],
                    ones_row=konst[0:1, 6 * P:7 * P],
                    b_row=konst[0:1, 7 * P:8 * P],
                    t_out=t_ou, pool_sfx="u", GRP=GRP_U,
                )
                # item phase: relations buys (src=user) + tags (src=tag)
                phase(
                    res_i, msg_specs=specs_i,
                    t_xd=t_xdi,
                    wl_list=[konst[:, 3 * P:4 * P], konst[:, 4 * P:5 * P]],
                    wr_col=konst[:, 5 * P:6 * P],
                    ones_row=konst[0:1, 6 * P:7 * P],
                    b_row=konst[0:1, 8 * P:9 * P],
                    t_out=t_oi, pool_sfx="i", GRP=GRP_I,
                )

    nc.compile()
    return nc


# ------------------------------------------------------------------- kernel()

def kernel(x_user, x_item, x_tag, ei_buys, ei_rev, ei_tags,
           Wl_buys, Wr_buys, b_buys,
           Wl_rev, Wr_rev, b_rev,
           Wl_tags, Wr_tags, b_tags):
    from concourse import bass_utils

    x_user = np.asarray(x_user, np.float32)
    x_item = np.asarray(x_item, np.float32)
    x_tag = np.asarray(x_tag, np.float32)
    xu16 = np.ascontiguousarray(x_user.astype(np.float16))
    xi16 = np.ascontiguousarray(x_item.astype(np.float16))
    xt16 = np.ascontiguousarray(x_tag.astype(np.float16))
    ei_buys = np.asarray(ei_buys, np.int64)
    ei_rev = np.asarray(ei_rev, np.int64)
    ei_tags = np.asarray(ei_tags, np.int64)

    n_user, n_item, n_tag = x_user.shape[0], x_item.shape[0], x_tag.shape[0]
    C = NC_CORES
    ru, ri = n_user // C, n_item // C

    # degree counts + reciprocals per relation (over full dst domain)
    cnt_buys = np.bincount(ei_buys[1], minlength=n_item)
    cnt_rev = np.bincount(ei_rev[1], minlength=n_user)
    cnt_tags = np.bincount(ei_tags[1], minlength=n_item)
    r_buys = (0.5 / np.maximum(cnt_buys, 1)).astype(np.float32)
    r_rev = (1.0 / np.maximum(cnt_rev, 1)).astype(np.float32)
    r_tags = (0.5 / np.maximum(cnt_tags, 1)).astype(np.float32)

    # per-dst-row per-chunk counts for binning
    ch_rev = np.bincount(ei_rev[1] * 4 + ei_rev[0] // S_CHUNK,
                         minlength=n_user * 4).reshape(n_user, 4)
    ch_buys = np.bincount(ei_buys[1] * 4 + ei_buys[0] // S_CHUNK,
                          minlength=n_item * 4).reshape(n_item, 4)

    configs = _CAP_CONFIGS
    m_rev = m_buys = m_tags = None
    for (NHu, NHi, caph4, capl4, caph1, capl1) in configs:
        NHu, NHi = min(NHu, NW), min(NHi, NW)
        cap4u = np.array([caph4] * NHu + [capl4] * (NW - NHu), np.int64)
        cap4i = np.array([caph4] * NHi + [capl4] * (NW - NHi), np.int64)
        cap1w = np.array([caph1] * NHi + [capl1] * (NW - NHi), np.int64)
        ok = True
        win_u = np.empty(n_user, np.int64)
        pos_u = np.empty(n_user, np.int64)
        win_i = np.empty(n_item, np.int64)
        pos_i = np.empty(n_item, np.int64)
        wrows_u = np.empty((C, NW, P), np.int64)
        wrows_i = np.empty((C, NW, P), np.int64)
        caps_u = np.repeat(cap4u[:, None], 4, axis=1)
        caps_i = np.concatenate(
            [np.repeat(cap4i[:, None], 4, axis=1), cap1w[:, None]], axis=1)
        for c in range(C):
            r = _bin_node_type(ch_rev[c * ru:(c + 1) * ru], caps_u)
            if r is None:
                ok = False
                break
            win_u[c * ru:(c + 1) * ru] = r[0]
            pos_u[c * ru:(c + 1) * ru] = r[1]
            wrows_u[c] = r[2]
            cm = np.concatenate(
                [ch_buys[c * ri:(c + 1) * ri],
                 cnt_tags[c * ri:(c + 1) * ri][:, None]], axis=1)
            r = _bin_node_type(cm, caps_i)
            if r is None:
                ok = False
                break
            win_i[c * ri:(c + 1) * ri] = r[0]
            pos_i[c * ri:(c + 1) * ri] = r[1]
            wrows_i[c] = r[2]
        if not ok:
            continue
        m_rev = _edge_meta(ei_rev[0], ei_rev[1], n_user, win_u, pos_u,
                           r_rev, 4, cap4u)
        m_buys = _edge_meta(ei_buys[0], ei_buys[1], n_item, win_i, pos_i,
                            r_buys, 4, cap4i)
        m_tags = _edge_meta(ei_tags[0], ei_tags[1], n_item, win_i, pos_i,
                            r_tags, 1, cap1w)
        if m_rev is not None and m_buys is not None and m_tags is not None:
            break
    assert m_rev is not None and m_buys is not None and m_tags is not None, \
        "binning failed for all capacity configs"
    ntk_rev = tuple(int(x) // P for x in cap4u)
    ntk_buys = tuple(int(x) // P for x in cap4i)
    ntk1 = tuple(int(x) // P for x in cap1w)
    gi_rev, dl_rev, rc_rev = m_rev
    gi_buys, dl_buys, rc_buys = m_buys
    gi_tags, dl_tags, rc_tags = m_tags

    # host-permuted x_dst tables: row (w*128+pos) = slice row wrows[w, pos]
    def xd_perm(x16, wrows, c, rows_slice):
        v = wrows[c].reshape(-1).copy()
        v[v < 0] = 0
        return np.ascontiguousarray(
            x16[c * rows_slice + v])

    # constants: iota | Wl_rev | Wr_rev | Wlb | Wlt | Wr_it | ones | b_u | b_i
    iota = np.tile(np.arange(P, dtype=np.float32), (P, 1))
    ones_blk = np.zeros((P, P), np.float32)
    ones_blk[0, :] = 1.0
    bu_blk = np.zeros((P, P), np.float32)
    bu_blk[0, :] = np.asarray(b_rev, np.float32)
    bi_blk = np.zeros((P, P), np.float32)
    bi_blk[0, :] = 0.5 * (np.asarray(b_buys, np.float32)
                          + np.asarray(b_tags, np.float32))
    konst = np.concatenate([
        iota,
        np.asarray(Wl_rev, np.float32), np.asarray(Wr_rev, np.float32),
        np.asarray(Wl_buys, np.float32), np.asarray(Wl_tags, np.float32),
        0.5 * (np.asarray(Wr_buys, np.float32)
               + np.asarray(Wr_tags, np.float32)),
        ones_blk, bu_blk, bi_blk,
    ], axis=1).astype(np.float16)

    key = (ntk_rev, ntk_buys, ntk1, n_user, n_item, n_tag)
    if key not in _COMPILED_CACHE:
        _COMPILED_CACHE[key] = _build_program(*key)
    nc = _COMPILED_CACHE[key]

    def f16m(dl, rc):
        # fp16 planes [-dl | rc]: oh = relu(rc - rc*(iota - dl)^2)
        return np.ascontiguousarray(
            np.concatenate([-dl, rc], axis=1).astype(np.float16))

    def f32s(rc):
        return np.ascontiguousarray((-rc).astype(np.float32))

    in_maps = []
    for c in range(C):
        in_maps.append(dict(
            xu=xu16, xi=xi16, xt=xt16,
            xdu=xd_perm(xu16, wrows_u, c, ru),
            xdi=xd_perm(xi16, wrows_i, c, ri),
            konst=konst,
            gi_rev=gi_rev[c], gi_buys=gi_buys[c], gi_tags=gi_tags[c, 0],
            meta_rev=f16m(dl_rev[c], rc_rev[c]),
            meta_buys=f16m(dl_buys[c], rc_buys[c]),
            meta_tags=f16m(dl_tags[c], rc_tags[c]),
            ms_rev=f32s(rc_rev[c]), ms_buys=f32s(rc_buys[c]),
            ms_tags=f32s(rc_tags[c]),
        ))

    res = bass_utils.run_bass_kernel_spmd(
        nc, in_maps, core_ids=list(range(C)))

    out_user = np.empty((n_user, P), np.float32)
    out_item = np.empty((n_item, P), np.float32)
    for c in range(C):
        ou = res.results[c]["out_user"]
        oi = res.results[c]["out_item"]
        ru_rows = wrows_u[c].reshape(-1)
        ri_rows = wrows_i[c].reshape(-1)
        mu = ru_rows >= 0
        mi = ri_rows >= 0
        out_user[c * ru + ru_rows[mu]] = ou[mu]
        out_item[c * ri + ri_rows[mi]] = oi[mi]
    return out_user, out_item


# revision 18
# speedup vs baseline: 1.2909x; 1.0040x over previous
"""Bass/Trainium2 kernel for a heterogeneous-graph SAGEConv layer (DBGNNLayer).

Strategy: shard by DESTINATION node across the 8 cores (12,500 dst rows of
each node type per core) so no cross-core collectives are needed.  Within a
core, dst rows are packed into 100 windows of 128 rows each, using
load-balanced binning so that every (window, src-chunk) edge segment fits a
fixed capacity (SPMD-uniform static shapes).  Edge source rows are fetched
with dma_gather (int16 indices -> the 100k-row tables are split into 4
chunks of 25k rows) in fp16 (halves HBM gather bytes vs f32).  The
per-window segment mean is computed as a matmul with a scaled one-hot
matrix built on the vector engine in fp16:
    psum_msgT[fin, dstlocal] += Xg_tile[e, fin]^T @ onehot[e, dstlocal]
where onehot[e, d] = (iota[d] == dst_local[e]) * recip[e], recip folding in
the 1/deg mean and the HeteroConv 0.5.  The root term x_dst @ Wr reads a
host-pre-permuted x_dst table with a transposing HWDGE DMA (no gather, no
PE transpose).  The bias is folded into the PSUM accumulation as a rank-1
matmul (ones ⊗ b).  Final per window:
    out[dst, f] = msgT.T @ Wl (+ msgT_tags.T @ Wl_tags) + xdT.T @ Wr + 1⊗b
"""

import sys
import time

sys.path.insert(0, "/opt/trn_rl_repo")

import numpy as np

P = 128                 # partitions / feature dim / window rows
NC_CORES = 8
NW = 100                # windows per node type per core
S_CHUNK = 25000         # rows per gather chunk (int16-safe)
GRP_U = 20              # windows per gather group, user phase
GRP_I = 10              # windows per gather group, item phase

_COMPILED_CACHE = {}

# classed per-window capacities: NH heavy windows, NW-NH light.
# (NH, caph4, capl4, caph1, capl1): rev/buys per-chunk caps; tags caps.
# (NH_user, NH_item, caph4, capl4, caph1, capl1)
_CAP_CONFIGS = [
    (60, 70, 384, 256, 1408, 1024),   # tightest classed (preferred)
    (60, 70, 384, 256, 1408, 1152),   # verified classed
    (70, 80, 384, 256, 1408, 1152),   # looser classed
    (100, 100, 384, 384, 1280, 1280),  # uniform fallback
    (100, 100, 512, 512, 1408, 1408),  # enlarged fallback
]


# ----------------------------------------------------------------- host utils

def _wrap16(flat_idx):
    """[n] int -> [128, n//16] int16 wrapped in 16 partitions, replicated."""
    n = flat_idx.shape[0]
    assert n % 16 == 0
    base = flat_idx.reshape(n // 16, 16).T.astype(np.int16)  # [16, n//16]
    return np.tile(base, (8, 1))


def _pack_bins(count_vecs, caps_per_bin, nbins, rows_cap=P):
    """Assign rows to nbins bins (<=rows_cap rows each) s.t. per-coordinate
    load sums stay <= caps_per_bin[b].  Returns assignment [n] -> bin, None on
    failure.  caps_per_bin: [nbins, K]."""
    n, k = count_vecs.shape
    caps_per_bin = np.asarray(caps_per_bin, np.int64)
    totals = count_vecs.sum(1)
    order = np.argsort(-totals, kind="stable")
    # deal rows to bins proportionally to bin capacity: snake separately
    # within the heavy prefix and light suffix so the initial load tracks
    # each bin's cap.
    cap_tot = caps_per_bin.sum(1).astype(np.float64)
    share = cap_tot / cap_tot.sum()
    quota = np.round(share * n).astype(np.int64)
    while quota.sum() > n:
        quota[np.argmax(quota)] -= 1
    while quota.sum() < n:
        quota[np.argmin(quota)] += 1
    quota = np.minimum(quota, rows_cap)
    if quota.sum() < n:
        return None
    # snake across bins, skipping bins whose quota is exhausted
    assign = np.empty(n, np.int64)
    fill = np.zeros(nbins, np.int64)
    b = 0
    direction = 1
    for i in range(n):
        while fill[b] >= quota[b]:
            b += direction
            if b == nbins or b < 0:
                direction = -direction
                b += direction
        assign[order[i]] = b
        fill[b] += 1
        b += direction
        if b == nbins or b < 0:
            direction = -direction
            b += direction
    loads = np.zeros((nbins, k), np.int64)
    np.add.at(loads, assign, count_vecs)
    rows = np.bincount(assign, minlength=nbins)
    for _ in range(6000):
        over = loads - caps_per_bin
        bk = np.unravel_index(np.argmax(over), over.shape)
        if over[bk] <= 0:
            return assign
        b, ck = bk
        cand = np.where((assign == b) & (count_vecs[:, ck] > 0))[0]
        cand = cand[np.argsort(count_vecs[cand, ck])]
        slack = caps_per_bin[:, ck] - loads[:, ck]
        tgt_order = np.argsort(-slack, kind="stable")
        moved = False
        for tb in tgt_order:
            if rows[tb] >= rows_cap or tb == b or slack[tb] <= 0:
                continue
            # pick the largest mover that fits everywhere in tb
            for r in cand[::-1]:
                if np.all(loads[tb] + count_vecs[r] <= caps_per_bin[tb]):
                    assign[r] = tb
                    loads[b] -= count_vecs[r]
                    loads[tb] += count_vecs[r]
                    rows[b] -= 1
                    rows[tb] += 1
                    moved = True
                    break
            if moved:
                break
        if not moved:
            return None
    return None


def _pack_classed(count_vecs, nh, cap_h, cap_l, rows_cap=P):
    """Two-stage packer: split rows into heavy/light classes by total load,
    then LPT-balance each class across its uniform-cap bins."""
    n, k = count_vecs.shape
    nl = NW - nh
    cap_h = np.asarray(cap_h, np.float64)
    cap_l = np.asarray(cap_l, np.float64)
    order = np.argsort(-count_vecs.sum(1), kind="stable")
    split = None
    for q_l in (nl * rows_cap, int(nl * rows_cap * 0.98),
                int(nl * rows_cap * 0.95)):
        q_l = min(q_l, n)
        q_h = n - q_l
        if q_h > nh * rows_cap:
            continue
        if (count_vecs[order[q_h:]].sum(0) > cap_l * nl).any():
            continue
        if (count_vecs[order[:q_h]].sum(0) > cap_h * nh).any():
            continue
        split = q_h
        break
    if split is None:
        return None
    assign = np.empty(n, np.int64)

    def lpt(rows_idx, nbins, caps, bin0):
        loads = np.zeros((nbins, k))
        cnt = np.zeros(nbins, np.int64)
        capsafe = np.maximum(caps, 1)
        for i in rows_idx:
            v = count_vecs[i]
            fits = (cnt < rows_cap) & np.all(loads + v <= caps, axis=1)
            if not fits.any():
                return False
            rem = (caps - loads - v) / capsafe
            score = rem.min(axis=1) + 1e-4 * (rows_cap - cnt)
            score[~fits] = -np.inf
            b = int(np.argmax(score))
            assign[i] = bin0 + b
            loads[b] += v
            cnt[b] += 1
        return True

    if not lpt(order[:split], nh, cap_h, 0):
        return None
    if not lpt(order[split:], nl, cap_l, nh):
        return None
    return assign


def _bin_node_type(count_mat, caps_per_bin):
    """count_mat [12500, K]; returns (win_of [12500], pos_of [12500],
    wrows [NW,128] slice-local row id or -1)."""
    caps_arr = np.asarray(caps_per_bin)
    nh = int((caps_arr[:, 0] == caps_arr[0, 0]).sum()) \
        if (caps_arr[0, 0] != caps_arr[-1, 0]) else NW
    if nh < NW:
        assign = _pack_classed(count_mat, nh, caps_arr[0], caps_arr[-1])
    else:
        assign = None
    if assign is None:
        assign = _pack_bins(count_mat, caps_per_bin, NW)
    if assign is None:
        return None
    win_of = assign
    pos_of = np.empty_like(assign)
    wrows = -np.ones((NW, P), np.int64)
    for w in range(NW):
        rows = np.where(assign == w)[0]
        pos_of[rows] = np.arange(len(rows))
        wrows[w, : len(rows)] = rows
    return win_of, pos_of, wrows


def _edge_meta(src, dst, n_dst, win_of_all, pos_of_all, recip, n_chunks,
               capw):
    """Build per-core gather indices and per-tile metadata for one relation.

    capw: [NW] per-window per-chunk edge capacity (each a multiple of 128).
    Layout: idx16 [C, n_chunks, 128, TOT//16] where TOT = sum(capw); each
    chunk block is the window-major concat of capw[w] segments.
    dl/rc [C, 128, TCOL] where TCOL = n_chunks * sum(capw)//128; col =
    colbase[w] + k*ntile[w] + t, partition = edge position within tile.
    """
    C = NC_CORES
    capw = np.asarray(capw, np.int64)
    ntile_w = capw // P
    TOT = int(capw.sum())
    prefix = np.zeros(NW + 1, np.int64)
    np.cumsum(capw, out=prefix[1:])
    colbase = np.zeros(NW + 1, np.int64)
    np.cumsum(n_chunks * ntile_w, out=colbase[1:])
    TCOL = int(colbase[-1])

    rows_per_core = n_dst // C
    core = dst // rows_per_core
    k = src // S_CHUNK if n_chunks > 1 else np.zeros_like(src)
    w = win_of_all[dst]
    key = (core * NW + w) * n_chunks + k
    order = np.argsort(key, kind="stable")
    key_s = key[order]
    src_s = src[order]
    dst_s = dst[order]
    k_s = k[order]
    w_s = w[order]
    core_s = core[order]
    nseg = C * NW * n_chunks
    seg_counts = np.bincount(key, minlength=nseg)
    segcap = np.tile(np.repeat(capw, n_chunks), C)
    if (seg_counts > segcap).any():
        return None
    seg_start = np.zeros(nseg + 1, np.int64)
    np.cumsum(seg_counts, out=seg_start[1:])
    rank = np.arange(len(src)) - seg_start[key_s]
    # flat edge slot within [C][n_chunks][TOT]
    slot = (core_s * n_chunks + k_s) * TOT + prefix[w_s] + rank
    # flat meta position within [C][TCOL][P]
    mcol = colbase[w_s] + k_s * ntile_w[w_s] + rank // P
    mslot = (core_s * TCOL + mcol) * P + rank % P

    idx_pad = np.zeros(C * n_chunks * TOT, np.int64)
    dl_pad = np.full(C * TCOL * P, -1.0, np.float32)
    rc_pad = np.zeros(C * TCOL * P, np.float32)
    idx_pad[slot] = src_s - k_s * S_CHUNK
    dl_pad[mslot] = pos_of_all[dst_s]
    rc_pad[mslot] = recip[dst_s]

    idx_pad = idx_pad.reshape(C, n_chunks, TOT)
    idx16 = np.empty((C, n_chunks, 128, TOT // 16), np.int16)
    for c in range(C):
        for kk in range(n_chunks):
            idx16[c, kk] = _wrap16(idx_pad[c, kk])
    dl = dl_pad.reshape(C, TCOL, P).transpose(0, 2, 1)
    rc = rc_pad.reshape(C, TCOL, P).transpose(0, 2, 1)
    return np.ascontiguousarray(idx16), np.ascontiguousarray(dl), \
        np.ascontiguousarray(rc)


# ------------------------------------------------------------- device program

def _build_program(ntk_rev, ntk_buys, ntk1, n_user, n_item, n_tag):
    """ntk_*: tuple[NW] tiles/chunk per relation."""
    import concourse.bacc as bacc
    import concourse.bass as bass
    import concourse.mybir as mybir
    from concourse import tile

    f32 = mybir.dt.float32
    f16 = mybir.dt.float16
    i16 = mybir.dt.int16
    TOTR = sum(ntk_rev) * P
    TOTB = sum(ntk_buys) * P
    TOT1 = sum(ntk1) * P     # edges per tags block
    TCOLR = 4 * sum(ntk_rev)
    TCOLB = 4 * sum(ntk_buys)
    TCOL1 = sum(ntk1)

    nc = bacc.Bacc("TRN2", target_bir_lowering=False, debug=False,
                   enable_asserts=False, num_devices=NC_CORES)

    t_xu = nc.dram_tensor("xu", [n_user, P], f16, kind="ExternalInput")
    t_xi = nc.dram_tensor("xi", [n_item, P], f16, kind="ExternalInput")
    t_xt = nc.dram_tensor("xt", [n_tag, P], f16, kind="ExternalInput")
    # host-permuted x_dst tables (window-order rows), per core
    t_xdu = nc.dram_tensor("xdu", [NW * P, P], f16, kind="ExternalInput")
    t_xdi = nc.dram_tensor("xdi", [NW * P, P], f16, kind="ExternalInput")
    # konst: iota | Wl_rev | Wr_rev | b_rev | Wlb | Wlt | Wr_it | ones |
    #        b_u | b_i  (fp16, row0-only for the last three)
    t_const = nc.dram_tensor("konst", [P, 9 * P], f16, kind="ExternalInput")
    t_gi_rev = nc.dram_tensor("gi_rev", [4, 128, TOTR // 16], i16,
                              kind="ExternalInput")
    t_gi_buys = nc.dram_tensor("gi_buys", [4, 128, TOTB // 16], i16,
                               kind="ExternalInput")
    t_gi_tags = nc.dram_tensor("gi_tags", [128, TOT1 // 16], i16,
                               kind="ExternalInput")
    t_meta_rev = nc.dram_tensor("meta_rev", [P, 2 * TCOLR], f16,
                                kind="ExternalInput")
    t_meta_buys = nc.dram_tensor("meta_buys", [P, 2 * TCOLB], f16,
                                 kind="ExternalInput")
    t_meta_tags = nc.dram_tensor("meta_tags", [P, 2 * TCOL1], f16,
                                 kind="ExternalInput")
    t_ms_rev = nc.dram_tensor("ms_rev", [P, TCOLR], f32,
                              kind="ExternalInput")
    t_ms_buys = nc.dram_tensor("ms_buys", [P, TCOLB], f32,
                               kind="ExternalInput")
    t_ms_tags = nc.dram_tensor("ms_tags", [P, TCOL1], f32,
                               kind="ExternalInput")
    t_ou = nc.dram_tensor("out_user", [NW * P, P], f32, kind="ExternalOutput")
    t_oi = nc.dram_tensor("out_item", [NW * P, P], f32, kind="ExternalOutput")

    with tile.TileContext(nc) as tc:
        with tc.tile_pool(name="const", bufs=1) as cpool:
            konst = cpool.tile([P, 9 * P], f16)
            nc.sync.dma_start(konst[:], t_const.ap())
            iota = konst[:, 0:P]

            def load_resident(respool, msg_specs, sfx):
                """Load gather-index + meta tiles for a phase up front."""
                prefixes = []
                colbases = []
                for (t_gi, chunks, nch, ntks, t_meta, t_ms) in msg_specs:
                    pr = [0]
                    cb = [0]
                    for w in range(NW):
                        pr.append(pr[-1] + ntks[w] * P)
                        cb.append(cb[-1] + nch * ntks[w])
                    prefixes.append(pr)
                    colbases.append(cb)
                metas = []
                for si, (t_gi, chunks, nch, ntks, t_meta, t_ms) in \
                        enumerate(msg_specs):
                    mt = respool.tile([P, 2 * colbases[si][NW]], f16,
                                      tag=f"meta{sfx}{si}")
                    nc.sync.dma_start(mt[:], t_meta.ap())
                    ms = respool.tile([P, colbases[si][NW]], f32,
                                      tag=f"ms{sfx}{si}")
                    nc.sync.dma_start(ms[:], t_ms.ap())
                    metas.append((mt, ms))
                return prefixes, colbases, metas

            def phase(resident, msg_specs, t_xd, wl_list, wr_col, ones_row,
                      b_row, t_out, pool_sfx, GRP):
                GRP = min(GRP, NW)
                assert NW % GRP == 0
                bounds = list(range(0, NW + 1, GRP))
                GMAXW = GRP
                gsizes = [GRP] * (NW // GRP)
                """msg_specs: list of (t_gi, gather_chunks_list, n_chunks,
                ntk_list, t_meta)."""
                prefixes, colbases, metas = resident
                with tc.tile_pool(name="gx" + pool_sfx, bufs=1) as gxpool, \
                     tc.tile_pool(name="g" + pool_sfx, bufs=2) as gpool, \
                     tc.tile_pool(name="w" + pool_sfx, bufs=4) as wpool, \
                     tc.tile_pool(name="o" + pool_sfx, bufs=2) as opool, \
                     tc.tile_pool(name="p" + pool_sfx, bufs=2,
                                  space="PSUM") as ppool:
                    gidx_tiles = []
                    for si, (t_gi, chunks, nch, ntks, t_meta, t_ms) in \
                            enumerate(msg_specs):
                        cols = prefixes[si][NW] // 16
                        gt = gxpool.tile([128, nch * cols], i16,
                                         tag=f"gi{pool_sfx}{si}")
                        for kk in range(nch):
                            src_ap = t_gi.ap()[kk] if nch > 1 else t_gi.ap()
                            nc.sync.dma_start(
                                gt[:, kk * cols:(kk + 1) * cols], src_ap)
                        gidx_tiles.append(gt)

                    for g in range(len(gsizes)):
                        g0, g1 = bounds[g], bounds[g + 1]
                        # gathers for this window group
                        xg_bufs = []
                        for si, (t_gi, chunks, nch, ntks, t_meta, t_ms) in \
                                enumerate(msg_specs):
                            cols = prefixes[si][NW] // 16
                            e0, e1 = prefixes[si][g0], prefixes[si][g1]
                            ge = e1 - e0
                            gmax = max(
                                prefixes[si][bounds[a + 1]]
                                - prefixes[si][bounds[a]]
                                for a in range(len(gsizes)))
                            xg = gpool.tile([P, nch * gmax], f16,
                                            tag=f"xg{si}")
                            for kk in range(nch):
                                nc.gpsimd.dma_gather(
                                    out_ap=xg[:, kk * gmax:kk * gmax + ge]
                                    .rearrange("p (t f) -> p t f", f=P),
                                    in_ap=chunks[kk],
                                    idxs_ap=gidx_tiles[si][
                                        :, kk * cols + e0 // 16:
                                        kk * cols + e1 // 16],
                                    num_idxs=ge,
                                    num_idxs_reg=ge,
                                    elem_size=P,
                                    single_packet=False,
                                )
                            xg_bufs.append(xg)
                        # root rows, transposed on the fly: xdT[f, d]
                        xdT = gpool.tile([P, GMAXW * P], f16, tag="xd")
                        nc.sync.dma_start(
                            xdT[:, 0:(g1 - g0) * P],
                            t_xd.ap()[g0 * P:g1 * P, :],
                            transpose=True)

                        for wl_ in range(g1 - g0):
                            w = g0 + wl_
                            msg_sbs = []
                            for si, (t_gi, chunks, nch, ntks, t_meta,
                                     t_ms) in enumerate(msg_specs):
                                ntk = ntks[w]
                                TC = colbases[si][NW]
                                gmax = max(
                                    prefixes[si][bounds[a + 1]]
                                    - prefixes[si][bounds[a]]
                                    for a in range(len(gsizes)))
                                woff = (prefixes[si][w]
                                        - prefixes[si][g0]) // P
                                ps_msg = ppool.tile([P, P], f32,
                                                    space="PSUM",
                                                    tag=f"msg{si}")
                                for kk in range(nch):
                                    for t in range(ntk):
                                        col = (colbases[si][w]
                                               + kk * ntk + t)
                                        oh = wpool.tile([P, P], f16,
                                                        tag=f"oh{si}")
                                        # ACT: oh = relu(rc - rc*(iota-dl)^2)
                                        tmp = wpool.tile(
                                            [P, P], f16, tag=f"tmp{si}")
                                        mt, ms = metas[si]
                                        nc.scalar.activation(
                                            out=tmp[:], in_=iota,
                                            func=mybir.
                                            ActivationFunctionType.Square,
                                            bias=mt[:, col:col + 1],
                                            scale=1.0)
                                        nc.scalar.activation(
                                            out=oh[:], in_=tmp[:],
                                            func=mybir.
                                            ActivationFunctionType.Relu,
                                            bias=mt[:, TC + col:TC + col + 1],
                                            scale=ms[:, col:col + 1])
                                        xg = xg_bufs[si]
                                        tt = kk * (gmax // P) + woff + t
                                        nc.tensor.matmul(
                                            out=ps_msg[:],
                                            lhsT=xg[:, tt * P:(tt + 1) * P],
                                            rhs=oh[:],
                                            start=(kk == 0 and t == 0),
                                            stop=(kk == nch - 1
                                                  and t == ntk - 1),
                                        )
                                msg_sb = wpool.tile([P, P], f16,
                                                    tag=f"msgsb{si}")
                                nc.scalar.copy(out=msg_sb[:], in_=ps_msg[:])
                                msg_sbs.append(msg_sb)

                            ps_out = ppool.tile([P, P], f32, space="PSUM",
                                                tag="out")
                            for si, msg_sb in enumerate(msg_sbs):
                                nc.tensor.matmul(
                                    out=ps_out[:], lhsT=msg_sb[:],
                                    rhs=wl_list[si], start=(si == 0),
                                    stop=False)
                            nc.tensor.matmul(
                                out=ps_out[:],
                                lhsT=xdT[:, wl_ * P:(wl_ + 1) * P],
                                rhs=wr_col, start=False, stop=False)
                            # bias as rank-1: ones(row0) ⊗ b(row0)
                            nc.tensor.matmul(
                                out=ps_out[:], lhsT=ones_row, rhs=b_row,
                                start=False, stop=True)
                            out_sb = opool.tile([P, P], f32, tag="outsb")
                            nc.scalar.copy(out=out_sb[:], in_=ps_out[:])
                            nc.sync.dma_start(
                                t_out.ap()[w * P:(w + 1) * P, :], out_sb[:])

            xi_chunks = [t_xi.ap()[k * S_CHUNK:(k + 1) * S_CHUNK, :]
                         for k in range(4)]
            xu_chunks = [t_xu.ap()[k * S_CHUNK:(k + 1) * S_CHUNK, :]
                         for k in range(4)]
            specs_u = [(t_gi_rev, xi_chunks, 4, ntk_rev, t_meta_rev,
                        t_ms_rev)]
            specs_i = [
                (t_gi_buys, xu_chunks, 4, ntk_buys, t_meta_buys, t_ms_buys),
                (t_gi_tags, [t_xt.ap()], 1, ntk1, t_meta_tags, t_ms_tags),
            ]
            with tc.tile_pool(name="res", bufs=1) as respool:
                res_u = load_resident(respool, specs_u, "u")
                res_i = load_resident(respool, specs_i, "i")
                # user phase: relation rev (src=item)
                phase(
                    res_u, msg_specs=specs_u,
                    t_xd=t_xdu,
                    wl_list=[konst[:, 1 * P:2 * P]],
                    wr_col=konst[:, 2 * P:3 * P],
                    ones_row=konst[0:1, 6 * P:7 * P],
                    b_row=konst[0:1, 7 * P:8 * P],
                    t_out=t_ou, pool_sfx="u", GRP=GRP_U,
                )
                # item phase: relations buys (src=user) + tags (src=tag)
                phase(
                    res_i, msg_specs=specs_i,
                    t_xd=t_xdi,
                    wl_list=[konst[:, 3 * P:4 * P], konst[:, 4 * P:5 * P]],
                    wr_col=konst[:, 5 * P:6 * P],
                    ones_row=konst[0:1, 6 * P:7 * P],
                    b_row=konst[0:1, 8 * P:9 * P],
                    t_out=t_oi, pool_sfx="i", GRP=GRP_I,
                )

    nc.compile()
    return nc


# ------------------------------------------------------------------- kernel()

def kernel(x_user, x_item, x_tag, ei_buys, ei_rev, ei_tags,
           Wl_buys, Wr_buys, b_buys,
           Wl_rev, Wr_rev, b_rev,
           Wl_tags, Wr_tags, b_tags):
    from concourse import bass_utils

    x_user = np.asarray(x_user, np.float32)
    x_item = np.asarray(x_item, np.float32)
    x_tag = np.asarray(x_tag, np.float32)
    xu16 = np.ascontiguousarray(x_user.astype(np.float16))
    xi16 = np.ascontiguousarray(x_item.astype(np.float16))
    xt16 = np.ascontiguousarray(x_tag.astype(np.float16))
    ei_buys = np.asarray(ei_buys, np.int64)
    ei_rev = np.asarray(ei_rev, np.int64)
    ei_tags = np.asarray(ei_tags, np.int64)

    n_user, n_item, n_tag = x_user.shape[0], x_item.shape[0], x_tag.shape[0]
    C = NC_CORES
    ru, ri = n_user // C, n_item // C

    # degree counts + reciprocals per relation (over full dst domain)
    cnt_buys = np.bincount(ei_buys[1], minlength=n_item)
    cnt_rev = np.bincount(ei_rev[1], minlength=n_user)
    cnt_tags = np.bincount(ei_tags[1], minlength=n_item)
    r_buys = (0.5 / np.maximum(cnt_buys, 1)).astype(np.float32)
    r_rev = (1.0 / np.maximum(cnt_rev, 1)).astype(np.float32)
    r_tags = (0.5 / np.maximum(cnt_tags, 1)).astype(np.float32)

    # per-dst-row per-chunk counts for binning
    ch_rev = np.bincount(ei_rev[1] * 4 + ei_rev[0] // S_CHUNK,
                         minlength=n_user * 4).reshape(n_user, 4)
    ch_buys = np.bincount(ei_buys[1] * 4 + ei_buys[0] // S_CHUNK,
                          minlength=n_item * 4).reshape(n_item, 4)

    configs = _CAP_CONFIGS
    m_rev = m_buys = m_tags = None
    for (NHu, NHi, caph4, capl4, caph1, capl1) in configs:
        NHu, NHi = min(NHu, NW), min(NHi, NW)
        cap4u = np.array([caph4] * NHu + [capl4] * (NW - NHu), np.int64)
        cap4i = np.array([caph4] * NHi + [capl4] * (NW - NHi), np.int64)
        cap1w = np.array([caph1] * NHi + [capl1] * (NW - NHi), np.int64)
        ok = True
        win_u = np.empty(n_user, np.int64)
        pos_u = np.empty(n_user, np.int64)
        win_i = np.empty(n_item, np.int64)
        pos_i = np.empty(n_item, np.int64)
        wrows_u = np.empty((C, NW, P), np.int64)
        wrows_i = np.empty((C, NW, P), np.int64)
        caps_u = np.repeat(cap4u[:, None], 4, axis=1)
        caps_i = np.concatenate(
            [np.repeat(cap4i[:, None], 4, axis=1), cap1w[:, None]], axis=1)
        for c in range(C):
            r = _bin_node_type(ch_rev[c * ru:(c + 1) * ru], caps_u)
            if r is None:
                ok = False
                break
            win_u[c * ru:(c + 1) * ru] = r[0]
            pos_u[c * ru:(c + 1) * ru] = r[1]
            wrows_u[c] = r[2]
            cm = np.concatenate(
                [ch_buys[c * ri:(c + 1) * ri],
                 cnt_tags[c * ri:(c + 1) * ri][:, None]], axis=1)
            r = _bin_node_type(cm, caps_i)
            if r is None:
                ok = False
                break
            win_i[c * ri:(c + 1) * ri] = r[0]
            pos_i[c * ri:(c + 1) * ri] = r[1]
            wrows_i[c] = r[2]
        if not ok:
            continue
        m_rev = _edge_meta(ei_rev[0], ei_rev[1], n_user, win_u, pos_u,
                           r_rev, 4, cap4u)
        m_buys = _edge_meta(ei_buys[0], ei_buys[1], n_item, win_i, pos_i,
                            r_buys, 4, cap4i)
        m_tags = _edge_meta(ei_tags[0], ei_tags[1], n_item, win_i, pos_i,
                            r_tags, 1, cap1w)
        if m_rev is not None and m_buys is not None and m_tags is not None:
            break
    assert m_rev is not None and m_buys is not None and m_tags is not None, \
        "binning failed for all capacity configs"
    ntk_rev = tuple(int(x) // P for x in cap4u)
    ntk_buys = tuple(int(x) // P for x in cap4i)
    ntk1 = tuple(int(x) // P for x in cap1w)
    gi_rev, dl_rev, rc_rev = m_rev
    gi_buys, dl_buys, rc_buys = m_buys
    gi_tags, dl_tags, rc_tags = m_tags

    # host-permuted x_dst tables: row (w*128+pos) = slice row wrows[w, pos]
    def xd_perm(x16, wrows, c, rows_slice):
        v = wrows[c].reshape(-1).copy()
        v[v < 0] = 0
        return np.ascontiguousarray(
            x16[c * rows_slice + v])

    # constants: iota | Wl_rev | Wr_rev | Wlb | Wlt | Wr_it | ones | b_u | b_i
    iota = np.tile(np.arange(P, dtype=np.float32), (P, 1))
    ones_blk = np.zeros((P, P), np.float32)
    ones_blk[0, :] = 1.0
    bu_blk = np.zeros((P, P), np.float32)
    bu_blk[0, :] = np.asarray(b_rev, np.float32)
    bi_blk = np.zeros((P, P), np.float32)
    bi_blk[0, :] = 0.5 * (np.asarray(b_buys, np.float32)
                          + np.asarray(b_tags, np.float32))
    konst = np.concatenate([
        iota,
        np.asarray(Wl_rev, np.float32), np.asarray(Wr_rev, np.float32),
        np.asarray(Wl_buys, np.float32), np.asarray(Wl_tags, np.float32),
        0.5 * (np.asarray(Wr_buys, np.float32)
               + np.asarray(Wr_tags, np.float32)),
        ones_blk, bu_blk, bi_blk,
    ], axis=1).astype(np.float16)

    key = (ntk_rev, ntk_buys, ntk1, n_user, n_item, n_tag)
    if key not in _COMPILED_CACHE:
        _COMPILED_CACHE[key] = _build_program(*key)
    nc = _COMPILED_CACHE[key]

    def f16m(dl, rc):
        # fp16 planes [-dl | rc]: oh = relu(rc - rc*(iota - dl)^2)
        return np.ascontiguousarray(
            np.concatenate([-dl, rc], axis=1).astype(np.float16))

    def f32s(rc):
        return np.ascontiguousarray((-rc).astype(np.float32))

    in_maps = []
    for c in range(C):
        in_maps.append(dict(
            xu=xu16, xi=xi16, xt=xt16,
            xdu=xd_perm(xu16, wrows_u, c, ru),
            xdi=xd_perm(xi16, wrows_i, c, ri),
            konst=konst,
            gi_rev=gi_rev[c], gi_buys=gi_buys[c], gi_tags=gi_tags[c, 0],
            meta_rev=f16m(dl_rev[c], rc_rev[c]),
            meta_buys=f16m(dl_buys[c], rc_buys[c]),
            meta_tags=f16m(dl_tags[c], rc_tags[c]),
            ms_rev=f32s(rc_rev[c]), ms_buys=f32s(rc_buys[c]),
            ms_tags=f32s(rc_tags[c]),
        ))

    res = bass_utils.run_bass_kernel_spmd(
        nc, in_maps, core_ids=list(range(C)))

    out_user = np.empty((n_user, P), np.float32)
    out_item = np.empty((n_item, P), np.float32)
    for c in range(C):
        ou = res.results[c]["out_user"]
        oi = res.results[c]["out_item"]
        ru_rows = wrows_u[c].reshape(-1)
        ri_rows = wrows_i[c].reshape(-1)
        mu = ru_rows >= 0
        mi = ri_rows >= 0
        out_user[c * ru + ru_rows[mu]] = ou[mu]
        out_item[c * ri + ri_rows[mi]] = oi[mi]
    return out_user, out_item
